# revision 1
# baseline (speedup 1.0000x reference)
"""GAT (3-layer, DGL-style) on 8 Trainium2 NeuronCores.

Sharding: nodes across the 8 cores (6250 each, padded to 6272 = 49*128),
per-core nodes permuted by descending in-degree.  A "window" is 128 nodes;
a node is pinned to one SBUF partition lane of its window.  Per layer:

  Phase A (node side): featT = W^T @ h^T per window on PE, el/er via a small
  second matmul, build gather-table rows [feat(128 f32) | el(H f32)] with a
  768B stride in local DRAM, AllGather the tables across cores.

  Phase B (edge side): per window, edge tiles of 128 edges = one in-edge per
  destination partition.  dma_gather fetches 768B source rows (int16 indices;
  the 50176-row table is indexed as two 25088-row halves, each window's tiles
  are grouped into lo-half then hi-half passes).  er[dst] is a per-partition
  constant.  exp(lrelu(s)-C) = max(exp(s-C), exp(0.2*s-C)) on ACT.  Messages
  (+ per-head exp columns) are segment-summed by an identity-lhsT PE matmul
  accumulating into one PSUM bank per window.

C is a per-core bound lrelu(max el + max er) + 3 computed on device; shifting
exp by C instead of the per-segment max changes the reference's +1e-9 epsilon
term by < 1e-3 relative.
"""

import os
import sys

sys.path.insert(0, "/opt/trn_rl_repo")

import numpy as np

import concourse.bass as bass
import concourse.bacc as bacc
import concourse.mybir as mybir
import concourse.tile as tile
from concourse import library_config
from concourse.bass_utils import run_bass_kernel_spmd

F32 = mybir.dt.float32
I16 = mybir.dt.int16
AF = mybir.ActivationFunctionType
OP = mybir.AluOpType
AX = mybir.AxisListType

N_CORES = 8
DIM = 128
ROW_F32 = 192          # table row stride in f32 (768 B, multiple of 256 B)
TBL_COLS = 132         # used cols: 128 feat + up to 4 el slots
CAP = 16               # max tiles per dma_gather call
NEG_SLOPE = 0.2
C_MARGIN = 3.0
HEADS = (4, 4, 1)


# ---------------------------------------------------------------------------
# Host-side preprocessing
# ---------------------------------------------------------------------------

def preprocess(src, dst, n_nodes):
    src = np.asarray(src).astype(np.int64)
    dst = np.asarray(dst).astype(np.int64)
    npc = n_nodes // N_CORES
    NP = ((npc + 127) // 128) * 128
    W = NP // 128
    HALF = 4 * NP
    assert HALF <= 32768, HALF

    core = dst // npc
    local = dst - core * npc

    perm = []
    pos_of = np.empty(n_nodes, dtype=np.int64)
    for c in range(N_CORES):
        deg_c = np.bincount(local[core == c], minlength=npc)
        p = np.argsort(-deg_c, kind="stable")
        perm.append(p)
        inv = np.empty(npc, dtype=np.int64)
        inv[p] = np.arange(npc)
        pos_of[c * npc:(c + 1) * npc] = inv
    row_of = (np.arange(n_nodes) // npc) * NP + pos_of

    seg_pos = pos_of[dst]
    wv = seg_pos // 128
    pv = seg_pos % 128
    half = (row_of[src] >= HALF).astype(np.int64)

    # occurrence rank within (core, seg, half)
    key = (core * NP + seg_pos) * 2 + half
    order = np.argsort(key, kind="stable")
    ks = key[order]
    starts = np.r_[0, np.flatnonzero(np.diff(ks)) + 1]
    gid = np.zeros(len(ks), dtype=np.int64)
    gid[starts[1:]] = 1
    gid = np.cumsum(gid)
    t_in = np.arange(len(ks)) - starts[gid]
    tv = np.empty(len(ks), dtype=np.int64)
    tv[order] = t_in

    cnt = np.bincount(key, minlength=N_CORES * NP * 2).reshape(
        N_CORES, W, 128, 2)
    T_lo = cnt[:, :, :, 0].max(axis=(0, 2)).astype(np.int64)
    T_hi = cnt[:, :, :, 1].max(axis=(0, 2)).astype(np.int64)

    calls = []
    for w in range(W):
        for hf, T in ((0, int(T_lo[w])), (1, int(T_hi[w]))):
            t = 0
            while t < T:
                nt = min(CAP, T - t)
                calls.append((w, hf, nt))
                t += nt
    gtot = int(T_lo.sum() + T_hi.sum())
    icols = 8 * sum(nt for (_, _, nt) in calls)

    tile_off = np.zeros((W, 2), dtype=np.int64)
    acc = 0
    for w in range(W):
        tile_off[w, 0] = acc
        acc += T_lo[w]
        tile_off[w, 1] = acc
        acc += T_hi[w]

    idx_imgs, valids = [], []
    for c in range(N_CORES):
        m = core == c
        slots_idx = np.zeros((128, gtot), dtype=np.int64)
        slots_val = np.zeros((128, gtot), dtype=np.float32)
        g = tile_off[wv[m], half[m]] + tv[m]
        slots_idx[pv[m], g] = row_of[src[m]] - half[m] * HALF
        slots_val[pv[m], g] = 1.0
        img = np.zeros((16, icols), dtype=np.int16)
        colp = 0
        tile_ptr = {}
        for (w, hf, nt) in calls:
            t0 = tile_ptr.get((w, hf), 0)
            g0 = tile_off[w, hf] + t0
            part = slots_idx[:, g0:g0 + nt]          # [128, nt]
            flat = part.T.reshape(-1)                # j = t*128 + p
            img[:, colp:colp + nt * 8] = flat.reshape(nt * 8, 16).T
            colp += nt * 8
            tile_ptr[(w, hf)] = t0 + nt
        idx_imgs.append(np.ascontiguousarray(np.tile(img, (8, 1))))
        valids.append(slots_val)

    return dict(perm=perm, calls=calls, T_lo=T_lo, T_hi=T_hi,
                idx_img=idx_imgs, valid=valids, NP=NP, W=W, gtot=gtot,
                icols=icols, npc=npc, HALF=HALF,
                tile_off=tile_off)


def pack_weights(Wl, al, ar):
    H, Dh = Wl.shape[1], Wl.shape[2]
    Wm = np.ascontiguousarray(np.asarray(Wl, dtype=np.float32)
                              .reshape(Wl.shape[0], H * Dh))
    A = np.zeros((H * Dh, 8), dtype=np.float32)
    for h in range(H):
        A[h * Dh:(h + 1) * Dh, h] = np.asarray(al, dtype=np.float32)[h]
        A[h * Dh:(h + 1) * Dh, 4 + h] = np.asarray(ar, dtype=np.float32)[h]
    return Wm, A


# ---------------------------------------------------------------------------
# Device kernel
# ---------------------------------------------------------------------------

def build_nc(meta):
    NP, W, gtot, icols = meta["NP"], meta["W"], meta["gtot"], meta["icols"]
    calls, HALF = meta["calls"], meta["HALF"]
    NTOT = N_CORES * NP
    tile_off = meta["tile_off"]

    nc = bacc.Bacc(None, target_bir_lowering=False, debug=False,
                   num_devices=N_CORES, num_swdge_queues=4)

    hT0 = nc.declare_dram_parameter("hT0", [128, NP], F32, isOutput=False)
    idx_p = nc.declare_dram_parameter("idx", [128, icols], I16, isOutput=False)
    val_p = nc.declare_dram_parameter("valid", [128, gtot], F32,
                                      isOutput=False)
    Wp = [nc.declare_dram_parameter(f"W{l}", [128, 128], F32, isOutput=False)
          for l in range(3)]
    Ap = [nc.declare_dram_parameter(f"A{l}", [128, 8], F32, isOutput=False)
          for l in range(3)]
    ident_p = nc.declare_dram_parameter("ident", [128, 128], F32,
                                        isOutput=False)
    ones_p = nc.declare_dram_parameter("ones1", [1, 128], F32, isOutput=False)
    onescol_p = nc.declare_dram_parameter("onescol", [128, 1], F32,
                                          isOutput=False)
    out_p = nc.declare_dram_parameter("out", [NP, 128], F32, isOutput=True)

    with tile.TileContext(nc) as tc:
        with (
            tc.tile_pool(name="const", bufs=1) as constp,
            tc.tile_pool(name="persist", bufs=1) as pers,
            tc.tile_pool(name="featg", bufs=3) as fgp,
            tc.tile_pool(name="mext", bufs=3) as mxp,
            tc.tile_pool(name="small", bufs=4) as smp,
            tc.tile_pool(name="psum", bufs=3, space="PSUM") as psp,
            tc.tile_pool(name="psacc", bufs=2, space="PSUM") as psaccp,
            tc.tile_pool(name="dram", bufs=1, space="DRAM") as dramp,
        ):
            ident = constp.tile([128, 128], F32, tag="ident")
            nc.sync.dma_start(ident[:], ident_p[:, :])
            ones1 = constp.tile([1, 128], F32, tag="ones1")
            nc.sync.dma_start(ones1[:], ones_p[:, :])
            onescol = constp.tile([128, 1], F32, tag="onescol")
            nc.sync.dma_start(onescol[:], onescol_p[:, :])
            Wt = [constp.tile([128, 128], F32, tag=f"W{l}", name=f"Wt{l}") for l in range(3)]
            At = [constp.tile([128, 8], F32, tag=f"A{l}", name=f"At{l}") for l in range(3)]
            for l in range(3):
                nc.sync.dma_start(Wt[l][:], Wp[l][:, :])
                nc.sync.dma_start(At[l][:], Ap[l][:, :])
            idx_sb = pers.tile([128, icols], I16, tag="idx")
            nc.sync.dma_start(idx_sb[:], idx_p[:, :])
            valid_sb = pers.tile([128, gtot], F32, tag="valid")
            nc.sync.dma_start(valid_sb[:], val_p[:, :])

            hT = [pers.tile([128, W, 128], F32, tag=f"hT{i}", name=f"hT{i}")
                  for i in range(2)]
            nc.sync.dma_start(hT[0][:, :, :],
                              hT0[:, :].rearrange("p (w n) -> p w n", w=W))

            elerB = pers.tile([128, W, 8], F32, tag="elerB")
            rowimg = pers.tile([128, W, TBL_COLS], F32, tag="rowimg")

            loc_tbl = dramp.tile([NP, ROW_F32], F32, tag="loctbl")
            full_tbl = dramp.tile([NTOT, ROW_F32], F32, tag="fulltbl")
            zpad = smp.tile([128, ROW_F32 - TBL_COLS], F32, tag="zpad")
            nc.vector.memset(zpad[:], 0.0)
            for w in range(W):
                nc.sync.dma_start(
                    loc_tbl[:].rearrange("(w p) f -> w p f", p=128)
                    [w, :, TBL_COLS:ROW_F32],
                    zpad[:])


            CUT = os.environ.get("KGAT_CUT", "")
            n_layers = 1 if CUT else 3
            for layer in range(n_layers):
                H = HEADS[layer]
                D = 128 // H
                hcur, hnext = hT[layer % 2], hT[(layer + 1) % 2]

                # ======== Phase A ========
                if CUT == "B":
                    nc.vector.memset(rowimg[:, :, 0:TBL_COLS], 0.5)
                    nc.vector.memset(elerB[:, :, :], 0.1)
                for w in ([] if CUT == "B" else range(W)):
                    featT_ps = psp.tile([128, 128], F32, tag="ps")
                    nc.tensor.matmul(featT_ps[:], Wt[layer][:],
                                     hcur[:, w, :], start=True, stop=True)
                    featT_sb = smp.tile([128, 128], F32, tag="featT_sb")
                    nc.vector.tensor_copy(featT_sb[:], featT_ps[:])
                    elerT_ps = psp.tile([8, 128], F32, tag="ps")
                    nc.tensor.matmul(elerT_ps[:], At[layer][:], featT_sb[:],
                                     start=True, stop=True)
                    elerT_sb = smp.tile([8, 128], F32, tag="elerT_sb")
                    nc.vector.tensor_copy(elerT_sb[:], elerT_ps[:])
                    eler_ps = psp.tile([128, 8], F32, tag="ps")
                    nc.tensor.matmul(eler_ps[:], elerT_sb[:],
                                     ident[0:8, 0:8], is_transpose=True,
                                     start=True, stop=True)
                    nc.vector.tensor_copy(elerB[:, w, :], eler_ps[:])
                    feat_ps = psp.tile([128, 128], F32, tag="ps")
                    nc.tensor.matmul(feat_ps[:], featT_sb[:], ident[:, :],
                                     is_transpose=True, start=True, stop=True)
                    nc.vector.tensor_copy(rowimg[:, w, 0:128], feat_ps[:])
                    nc.vector.tensor_copy(rowimg[:, w, 128:128 + H],
                                          eler_ps[:, 0:H])
                    nc.sync.dma_start(
                        loc_tbl[:].rearrange("(w p) f -> w p f", p=128)
                        [w, :, 0:TBL_COLS],
                        rowimg[:, w, :])
                if CUT == "B":
                    for w in range(W):
                        nc.sync.dma_start(
                            loc_tbl[:].rearrange("(w p) f -> w p f", p=128)
                            [w, :, 0:TBL_COLS],
                            rowimg[:, w, :])

                # ---- AllGather ----
                nc.gpsimd.collective_compute(
                    "AllGather", OP.bypass,
                    replica_groups=[list(range(N_CORES))],
                    ins=[loc_tbl[:].opt()], outs=[full_tbl[:].opt()])

                # ---- -C = -(lrelu(max el + max er) + margin) ----
                if CUT == "B":
                    negC = smp.tile([128, 1], F32, tag="negC")
                    nc.vector.memset(negC[:], -1.0)
                else:
                    mx = smp.tile([128, 2], F32, tag="mx")
                    nc.vector.tensor_reduce(mx[:, 0:1], elerB[:, :, 0:H],
                                            axis=AX.XY, op=OP.max)
                    nc.vector.tensor_reduce(mx[:, 1:2], elerB[:, :, 4:4 + H],
                                            axis=AX.XY, op=OP.max)
                    mxT_ps = psp.tile([2, 128], F32, tag="ps")
                    nc.tensor.matmul(mxT_ps[:], mx[:], ident[:, :],
                                     is_transpose=True, start=True, stop=True)
                    mm = smp.tile([2, 1], F32, tag="mm")
                    nc.vector.tensor_reduce(mm[:], mxT_ps[:, :], axis=AX.X,
                                            op=OP.max)
                    s_ps = psp.tile([1, 1], F32, tag="ps")
                    nc.tensor.matmul(s_ps[:], mm[:], onescol[0:2, 0:1],
                                     start=True, stop=True)
                    cs = smp.tile([1, 4], F32, tag="cs")
                    nc.vector.tensor_copy(cs[:, 0:1], s_ps[:])
                    nc.vector.tensor_scalar(cs[:, 1:2], cs[:, 0:1], NEG_SLOPE,
                                            None, op0=OP.mult)
                    nc.vector.tensor_tensor(cs[:, 2:3], cs[:, 0:1],
                                            cs[:, 1:2], op=OP.max)
                    nc.vector.tensor_scalar(cs[:, 3:4], cs[:, 2:3], -1.0,
                                            -C_MARGIN, op0=OP.mult,
                                            op1=OP.add)
                    negC_ps = psp.tile([128, 1], F32, tag="ps")
                    nc.tensor.matmul(negC_ps[:], ones1[:], cs[:, 3:4],
                                     start=True, stop=True)
                    negC = smp.tile([128, 1], F32, tag="negC")
                    nc.vector.tensor_copy(negC[:], negC_ps[:])

                # ======== Phase B ========
                tbl_lo = full_tbl[0:HALF, :]
                tbl_hi = full_tbl[HALF:NTOT, :]
                colp = 0
                tile_ptr = {}
                cur_w = -1
                acc_ps = None
                first_mm = True
                ntiles_w = {w: int(meta["T_lo"][w] + meta["T_hi"][w])
                            for w in range(W)}
                done_w = {w: 0 for w in range(W)}
                qn = 0
                for (w, hf, nt) in (calls if CUT != "A" else []):
                    if w != cur_w:
                        cur_w = w
                        acc_ps = psaccp.tile([128, TBL_COLS], F32, tag="acc")
                        first_mm = True
                    t0 = tile_ptr.get((w, hf), 0)
                    tile_ptr[(w, hf)] = t0 + nt
                    g0 = int(tile_off[w, hf]) + t0

                    fg = fgp.tile([128, CAP, ROW_F32], F32, tag="fg")
                    src_ap = tbl_lo if hf == 0 else tbl_hi
                    nc.gpsimd.dma_gather(
                        fg[:, 0:nt, :], src_ap,
                        idx_sb[:, colp:colp + nt * 8],
                        nt * 128, nt * 128, ROW_F32, elem_step=ROW_F32,
                        single_packet=False, queue_num=qn)
                    qn = (qn + 1) % 4
                    colp += nt * 8

                    t = 0
                    while t < nt and CUT not in ("AB", "B"):
                        g = min(4, nt - t)
                        sx = smp.tile([128, 4, 4], F32, tag="sx")
                        ux = smp.tile([128, 4, 4], F32, tag="ux")
                        ex = smp.tile([128, 4, 4], F32, tag="exx")
                        er_b = (elerB[:, w, 4:4 + H].unsqueeze(1)
                                .broadcast_to([128, g, H]))
                        nc.vector.tensor_tensor(
                            sx[:, 0:g, 0:H], fg[:, t:t + g, 128:128 + H],
                            er_b, op=OP.add)
                        nc.scalar.activation(ux[:, 0:g, 0:H], sx[:, 0:g, 0:H],
                                             AF.Exp, bias=negC[:, 0:1],
                                             scale=1.0)
                        nc.scalar.activation(ex[:, 0:g, 0:H], sx[:, 0:g, 0:H],
                                             AF.Exp, bias=negC[:, 0:1],
                                             scale=NEG_SLOPE)
                        val_b = (valid_sb[:, g0 + t:g0 + t + g].unsqueeze(2)
                                 .broadcast_to([128, g, H]))
                        nc.vector.scalar_tensor_tensor(
                            ex[:, 0:g, 0:H], ux[:, 0:g, 0:H], 1.0,
                            ex[:, 0:g, 0:H], op0=OP.mult, op1=OP.max)
                        nc.vector.tensor_tensor(ex[:, 0:g, 0:H],
                                                ex[:, 0:g, 0:H], val_b,
                                                op=OP.mult)
                        mext = mxp.tile([128, 4, TBL_COLS], F32, tag="mext")
                        ex_b = (ex[:, 0:g, 0:H].unsqueeze(3)
                                .broadcast_to([128, g, H, D]))
                        nc.vector.tensor_tensor(
                            mext[:, 0:g, 0:128]
                            .rearrange("p g (h d) -> p g h d", h=H),
                            fg[:, t:t + g, 0:128]
                            .rearrange("p g (h d) -> p g h d", h=H),
                            ex_b, op=OP.mult)
                        nc.vector.tensor_copy(mext[:, 0:g, 128:128 + H],
                                              ex[:, 0:g, 0:H])
                        for k in range(g):
                            done_w[w] += 1
                            nc.tensor.matmul(
                                acc_ps[:, 0:128 + H], ident[:, :],
                                mext[:, k, 0:128 + H],
                                start=first_mm,
                                stop=(done_w[w] == ntiles_w[w]))
                            first_mm = False
                        t += g

                    if CUT in ("AB", "ABC") and tile_ptr[(w, hf)] >= 0:
                        pass
                    if done_w[w] == ntiles_w[w] and not CUT:
                        dn = smp.tile([128, 8], F32, tag="dn")
                        nc.vector.tensor_scalar(dn[:, 0:H],
                                                acc_ps[:, 128:128 + H],
                                                1e-9, None, op0=OP.add)
                        nc.vector.reciprocal(dn[:, 4:4 + H], dn[:, 0:H])
                        hsb = smp.tile([128, 128], F32, tag="hsb")
                        rec_b = (dn[:, 4:4 + H].unsqueeze(2)
                                 .broadcast_to([128, H, D]))
                        nc.vector.tensor_tensor(
                            hsb[:].rearrange("p (h d) -> p h d", h=H),
                            acc_ps[:, 0:128]
                            .rearrange("p (h d) -> p h d", h=H),
                            rec_b, op=OP.mult)
                        if layer < 2:
                            hT_ps = psp.tile([128, 128], F32, tag="ps")
                            nc.tensor.matmul(hT_ps[:], hsb[:], ident[:, :],
                                             is_transpose=True,
                                             start=True, stop=True)
                            nc.scalar.activation(hnext[:, w, :], hT_ps[:],
                                                 AF.Relu)
                        else:
                            nc.sync.dma_start(
                                out_p[:, :].rearrange("(w p) f -> w p f",
                                                      p=128)[w, :, :],
                                hsb[:])
            if CUT:
                for w in range(W):
                    nc.sync.dma_start(
                        out_p[:, :].rearrange("(w p) f -> w p f", p=128)
                        [w, :, :],
                        rowimg[:, w, 0:128])
    nc.finalize()
    return nc


# ---------------------------------------------------------------------------
# Entry point
# ---------------------------------------------------------------------------

def kernel(features, src, dst, W0, al0, ar0, W1, al1, ar1, W2, al2, ar2):
    out, _ = run_gat(features, src, dst, W0, al0, ar0, W1, al1, ar1,
                     W2, al2, ar2, trace=False)
    return out


def run_gat(features, src, dst, W0, al0, ar0, W1, al1, ar1, W2, al2, ar2,
            trace=False):
    features = np.asarray(features, dtype=np.float32)
    n_nodes = features.shape[0]
    meta = preprocess(src, dst, n_nodes)
    NP, W, npc = meta["NP"], meta["W"], meta["npc"]

    Wm0, A0 = pack_weights(np.asarray(W0), al0, ar0)
    Wm1, A1 = pack_weights(np.asarray(W1), al1, ar1)
    Wm2, A2 = pack_weights(np.asarray(W2), al2, ar2)

    ident = np.eye(128, dtype=np.float32)
    ones1 = np.ones((1, 128), dtype=np.float32)
    onescol = np.ones((128, 1), dtype=np.float32)

    in_maps = []
    for c in range(N_CORES):
        h_c = np.zeros((NP, 128), dtype=np.float32)
        h_c[:npc] = features[c * npc:(c + 1) * npc][meta["perm"][c]]
        in_maps.append({
            "hT0": np.ascontiguousarray(h_c.T),
            "idx": meta["idx_img"][c],
            "valid": meta["valid"][c],
            "W0": Wm0, "W1": Wm1, "W2": Wm2,
            "A0": A0, "A1": A1, "A2": A2,
            "ident": ident, "ones1": ones1, "onescol": onescol,
        })

    nc = build_nc(meta)
    br = run_bass_kernel_spmd(nc, in_maps, list(range(N_CORES)), trace=trace)
    res = br.results

    out = np.empty((n_nodes, 128), dtype=np.float32)
    for c in range(N_CORES):
        o = np.asarray(res[c]["out"])
        out[c * npc:(c + 1) * npc] = o[np.argsort(meta["perm"][c])]
    return out, br



# revision 2
# speedup vs baseline: 1.1943x; 1.1943x over previous
"""GAT (3-layer, DGL-style) on 8 Trainium2 NeuronCores.

Sharding: nodes across the 8 cores (6250 each, padded to 6272 = 49*128),
per-core nodes permuted by descending in-degree.  A "window" is 128 nodes;
a node is pinned to one SBUF partition lane of its window.  Per layer:

  Phase A (node side): featT = W^T @ h^T per window on PE (bf16), er via a
  small second matmul, write 256-byte bf16 feature rows to local DRAM,
  AllGather the 12.8 MB table across cores.

  Phase B (edge side): per window, edge tiles of 128 edges = one in-edge per
  destination partition.  dma_gather fetches 256B bf16 rows (int16 indices;
  the 50176-row table is indexed as two 25088-row halves).  el[src] is
  recomputed per edge on the vector engine (dot with the al vector
  replicated across partitions); er[dst] is a per-partition constant.
  exp(lrelu(s)-C) = max(exp(s-C), exp(0.2*s-C)) on ACT.  Messages (+
  per-head exp columns) are segment-summed by an identity-lhsT bf16 PE
  matmul accumulating into one PSUM bank per window.

C is a per-core bound lrelu(max el + max er) + 3 computed on device; shifting
exp by C instead of the per-segment max changes the reference's +1e-9 epsilon
term by < 1e-3 relative.
"""

import sys

sys.path.insert(0, "/opt/trn_rl_repo")

import ml_dtypes
import numpy as np

import concourse.bass as bass
import concourse.bacc as bacc
import concourse.mybir as mybir
import concourse.tile as tile
from concourse.bass_utils import run_bass_kernel_spmd

F32 = mybir.dt.float32
BF16 = mybir.dt.bfloat16
I16 = mybir.dt.int16
AF = mybir.ActivationFunctionType
OP = mybir.AluOpType
AX = mybir.AxisListType

N_CORES = 8
DIM = 128
ROW = 128              # bf16 elems per table row (256 B)
CAP = 32               # max tiles per dma_gather call
NEG_SLOPE = 0.2
C_MARGIN = 3.0
HEADS = (4, 4, 1)
BF = ml_dtypes.bfloat16


# ---------------------------------------------------------------------------
# Host-side preprocessing
# ---------------------------------------------------------------------------

def preprocess(src, dst, n_nodes):
    src = np.asarray(src).astype(np.int64)
    dst = np.asarray(dst).astype(np.int64)
    npc = n_nodes // N_CORES
    NP = ((npc + 127) // 128) * 128
    W = NP // 128
    HALF = 4 * NP
    assert HALF <= 32768, HALF

    core = dst // npc
    local = dst - core * npc

    perm = []
    pos_of = np.empty(n_nodes, dtype=np.int64)
    for c in range(N_CORES):
        deg_c = np.bincount(local[core == c], minlength=npc)
        p = np.argsort(-deg_c, kind="stable")
        perm.append(p)
        inv = np.empty(npc, dtype=np.int64)
        inv[p] = np.arange(npc)
        pos_of[c * npc:(c + 1) * npc] = inv
    row_of = (np.arange(n_nodes) // npc) * NP + pos_of

    seg_pos = pos_of[dst]
    wv = seg_pos // 128
    pv = seg_pos % 128
    half = (row_of[src] >= HALF).astype(np.int64)

    # occurrence rank within (core, seg, half)
    key = (core * NP + seg_pos) * 2 + half
    order = np.argsort(key, kind="stable")
    ks = key[order]
    starts = np.r_[0, np.flatnonzero(np.diff(ks)) + 1]
    gid = np.zeros(len(ks), dtype=np.int64)
    gid[starts[1:]] = 1
    gid = np.cumsum(gid)
    t_in = np.arange(len(ks)) - starts[gid]
    tv = np.empty(len(ks), dtype=np.int64)
    tv[order] = t_in

    cnt = np.bincount(key, minlength=N_CORES * NP * 2).reshape(
        N_CORES, W, 128, 2)
    T_lo = cnt[:, :, :, 0].max(axis=(0, 2)).astype(np.int64)
    T_hi = cnt[:, :, :, 1].max(axis=(0, 2)).astype(np.int64)

    calls = []
    for w in range(W):
        for hf, T in ((0, int(T_lo[w])), (1, int(T_hi[w]))):
            t = 0
            while t < T:
                nt = min(CAP, T - t)
                calls.append((w, hf, nt))
                t += nt
    gtot = int(T_lo.sum() + T_hi.sum())
    icols = 8 * sum(nt for (_, _, nt) in calls)

    tile_off = np.zeros((W, 2), dtype=np.int64)
    acc = 0
    for w in range(W):
        tile_off[w, 0] = acc
        acc += T_lo[w]
        tile_off[w, 1] = acc
        acc += T_hi[w]

    idx_imgs, valids = [], []
    for c in range(N_CORES):
        m = core == c
        slots_idx = np.zeros((128, gtot), dtype=np.int64)
        slots_val = np.zeros((128, gtot), dtype=np.float32)
        g = tile_off[wv[m], half[m]] + tv[m]
        slots_idx[pv[m], g] = row_of[src[m]] - half[m] * HALF
        slots_val[pv[m], g] = 1.0
        img = np.zeros((16, icols), dtype=np.int16)
        colp = 0
        tile_ptr = {}
        for (w, hf, nt) in calls:
            t0 = tile_ptr.get((w, hf), 0)
            g0 = tile_off[w, hf] + t0
            part = slots_idx[:, g0:g0 + nt]          # [128, nt]
            flat = part.T.reshape(-1)                # j = t*128 + p
            img[:, colp:colp + nt * 8] = flat.reshape(nt * 8, 16).T
            colp += nt * 8
            tile_ptr[(w, hf)] = t0 + nt
        idx_imgs.append(np.ascontiguousarray(np.tile(img, (8, 1))))
        valids.append(slots_val)

    return dict(perm=perm, calls=calls, T_lo=T_lo, T_hi=T_hi,
                idx_img=idx_imgs, valid=valids, NP=NP, W=W, gtot=gtot,
                icols=icols, npc=npc, HALF=HALF,
                tile_off=tile_off)


def pack_weights(Wl, al, ar):
    H, Dh = Wl.shape[1], Wl.shape[2]
    Wm = np.ascontiguousarray(np.asarray(Wl, dtype=np.float32)
                              .reshape(Wl.shape[0], H * Dh))
    A = np.zeros((H * Dh, 8), dtype=np.float32)
    for h in range(H):
        A[h * Dh:(h + 1) * Dh, h] = np.asarray(al, dtype=np.float32)[h]
        A[h * Dh:(h + 1) * Dh, 4 + h] = np.asarray(ar, dtype=np.float32)[h]
    alv = np.asarray(al, dtype=np.float32).reshape(-1)  # [H*Dh] flat
    alv_rep = np.ascontiguousarray(np.tile(alv[None, :], (128, 1)))
    return Wm, A, alv_rep


# ---------------------------------------------------------------------------
# Device kernel
# ---------------------------------------------------------------------------

def build_nc(meta):
    NP, W, gtot, icols = meta["NP"], meta["W"], meta["gtot"], meta["icols"]
    calls, HALF = meta["calls"], meta["HALF"]
    NTOT = N_CORES * NP
    tile_off = meta["tile_off"]

    nc = bacc.Bacc(None, target_bir_lowering=False, debug=False,
                   num_devices=N_CORES, num_swdge_queues=4)

    hT0 = nc.declare_dram_parameter("hT0", [128, NP], BF16, isOutput=False)
    idx_p = nc.declare_dram_parameter("idx", [128, icols], I16, isOutput=False)
    val_p = nc.declare_dram_parameter("valid", [128, gtot], F32,
                                      isOutput=False)
    Wp = [nc.declare_dram_parameter(f"W{l}", [128, 128], BF16, isOutput=False)
          for l in range(3)]
    Ap = [nc.declare_dram_parameter(f"A{l}", [128, 8], BF16, isOutput=False)
          for l in range(3)]
    ALp = [nc.declare_dram_parameter(f"AL{l}", [128, 128], BF16,
                                     isOutput=False) for l in range(3)]
    ident_p = nc.declare_dram_parameter("ident", [128, 128], F32,
                                        isOutput=False)
    identb_p = nc.declare_dram_parameter("identb", [128, 128], BF16,
                                         isOutput=False)
    ones_p = nc.declare_dram_parameter("ones1", [1, 128], F32, isOutput=False)
    onescol_p = nc.declare_dram_parameter("onescol", [128, 1], F32,
                                          isOutput=False)
    out_p = nc.declare_dram_parameter("out", [NP, 128], F32, isOutput=True)

    with tile.TileContext(nc) as tc:
        with (
            tc.tile_pool(name="const", bufs=1) as constp,
            tc.tile_pool(name="persist", bufs=1) as pers,
            tc.tile_pool(name="featg", bufs=4) as fgp,
            tc.tile_pool(name="mext", bufs=4) as mxp,
            tc.tile_pool(name="eltmp", bufs=4) as elp,
            tc.tile_pool(name="small", bufs=4) as smp,
            tc.tile_pool(name="psum", bufs=3, space="PSUM") as psp,
            tc.tile_pool(name="psacc", bufs=2, space="PSUM") as psaccp,
            tc.tile_pool(name="dram", bufs=1, space="DRAM") as dramp,
        ):
            ident = constp.tile([128, 128], F32, tag="ident")
            nc.sync.dma_start(ident[:], ident_p[:, :])
            identb = constp.tile([128, 128], BF16, tag="identb")
            nc.sync.dma_start(identb[:], identb_p[:, :])
            ones1 = constp.tile([1, 128], F32, tag="ones1")
            nc.sync.dma_start(ones1[:], ones_p[:, :])
            onescol = constp.tile([128, 1], F32, tag="onescol")
            nc.sync.dma_start(onescol[:], onescol_p[:, :])
            Wt = [constp.tile([128, 128], BF16, tag=f"W{l}", name=f"Wt{l}")
                  for l in range(3)]
            At = [constp.tile([128, 8], BF16, tag=f"A{l}", name=f"At{l}")
                  for l in range(3)]
            ALt = [constp.tile([128, 128], BF16, tag=f"AL{l}", name=f"ALt{l}")
                   for l in range(3)]
            for l in range(3):
                nc.sync.dma_start(Wt[l][:], Wp[l][:, :])
                nc.sync.dma_start(At[l][:], Ap[l][:, :])
                nc.sync.dma_start(ALt[l][:], ALp[l][:, :])
            idx_sb = pers.tile([128, icols], I16, tag="idx")
            nc.sync.dma_start(idx_sb[:], idx_p[:, :])
            valid_sb = pers.tile([128, gtot], F32, tag="valid")
            nc.sync.dma_start(valid_sb[:], val_p[:, :])

            hT = [pers.tile([128, W, 128], BF16, tag=f"hT{i}", name=f"hT{i}")
                  for i in range(2)]
            nc.sync.dma_start(hT[0][:, :, :],
                              hT0[:, :].rearrange("p (w n) -> p w n", w=W))

            elerB = pers.tile([128, W, 8], F32, tag="elerB")
            rowimg = pers.tile([128, W, ROW], BF16, tag="rowimg")

            loc_tbl = dramp.tile([NP, ROW], BF16, tag="loctbl")
            full_tbl = dramp.tile([NTOT, ROW], BF16, tag="fulltbl")

            for layer in range(3):
                H = HEADS[layer]
                D = 128 // H
                hcur, hnext = hT[layer % 2], hT[(layer + 1) % 2]

                # ======== Phase A ========
                for w in range(W):
                    featT_ps = psp.tile([128, 128], F32, tag="ps")
                    nc.tensor.matmul(featT_ps[:], Wt[layer][:],
                                     hcur[:, w, :], start=True, stop=True)
                    featT_sb = smp.tile([128, 128], BF16, tag="featT_sb")
                    nc.vector.tensor_copy(featT_sb[:], featT_ps[:])
                    elerT_ps = psp.tile([8, 128], F32, tag="ps")
                    nc.tensor.matmul(elerT_ps[:], At[layer][:], featT_sb[:],
                                     start=True, stop=True)
                    elerT_sb = smp.tile([8, 128], F32, tag="elerT_sb")
                    nc.vector.tensor_copy(elerT_sb[:], elerT_ps[:])
                    eler_ps = psp.tile([128, 8], F32, tag="ps")
                    nc.tensor.matmul(eler_ps[:], elerT_sb[:],
                                     ident[0:8, 0:8], is_transpose=True,
                                     start=True, stop=True)
                    nc.vector.tensor_copy(elerB[:, w, :], eler_ps[:])
                    feat_ps = psp.tile([128, 128], BF16, tag="psb")
                    nc.tensor.matmul(feat_ps[:], featT_sb[:], identb[:, :],
                                     is_transpose=True, start=True, stop=True)
                    nc.vector.tensor_copy(rowimg[:, w, :], feat_ps[:])
                    nc.sync.dma_start(
                        loc_tbl[:].rearrange("(w p) f -> w p f", p=128)
                        [w, :, :],
                        rowimg[:, w, :])

                # ---- AllGather ----
                nc.gpsimd.collective_compute(
                    "AllGather", OP.bypass,
                    replica_groups=[list(range(N_CORES))],
                    ins=[loc_tbl[:].opt()], outs=[full_tbl[:].opt()])

                # ---- -C = -(lrelu(max el + max er) + margin) ----
                mx = smp.tile([128, 2], F32, tag="mx")
                nc.vector.tensor_reduce(mx[:, 0:1], elerB[:, :, 0:H],
                                        axis=AX.XY, op=OP.max)
                nc.vector.tensor_reduce(mx[:, 1:2], elerB[:, :, 4:4 + H],
                                        axis=AX.XY, op=OP.max)
                mxT_ps = psp.tile([2, 128], F32, tag="ps")
                nc.tensor.matmul(mxT_ps[:], mx[:], ident[:, :],
                                 is_transpose=True, start=True, stop=True)
                mm = smp.tile([2, 1], F32, tag="mm")
                nc.vector.tensor_reduce(mm[:], mxT_ps[:, :], axis=AX.X,
                                        op=OP.max)
                s_ps = psp.tile([1, 1], F32, tag="ps")
                nc.tensor.matmul(s_ps[:], mm[:], onescol[0:2, 0:1],
                                 start=True, stop=True)
                cs = smp.tile([1, 4], F32, tag="cs")
                nc.vector.tensor_copy(cs[:, 0:1], s_ps[:])
                nc.vector.tensor_scalar(cs[:, 1:2], cs[:, 0:1], NEG_SLOPE,
                                        None, op0=OP.mult)
                nc.vector.tensor_tensor(cs[:, 2:3], cs[:, 0:1],
                                        cs[:, 1:2], op=OP.max)
                nc.vector.tensor_scalar(cs[:, 3:4], cs[:, 2:3], -1.0,
                                        -C_MARGIN, op0=OP.mult,
                                        op1=OP.add)
                negC_ps = psp.tile([128, 1], F32, tag="ps")
                nc.tensor.matmul(negC_ps[:], ones1[:], cs[:, 3:4],
                                 start=True, stop=True)
                negC = smp.tile([128, 1], F32, tag="negC")
                nc.vector.tensor_copy(negC[:], negC_ps[:])

                # ======== Phase B ========
                tbl_lo = full_tbl[0:HALF, :]
                tbl_hi = full_tbl[HALF:NTOT, :]
                colp = 0
                tile_ptr = {}
                cur_w = -1
                acc_ps = None
                first_mm = True
                ntiles_w = {w: int(meta["T_lo"][w] + meta["T_hi"][w])
                            for w in range(W)}
                done_w = {w: 0 for w in range(W)}
                qn = 0
                for (w, hf, nt) in calls:
                    if w != cur_w:
                        cur_w = w
                        acc_ps = psaccp.tile([128, 132], F32, tag="acc")
                        first_mm = True
                    t0 = tile_ptr.get((w, hf), 0)
                    tile_ptr[(w, hf)] = t0 + nt
                    g0 = int(tile_off[w, hf]) + t0

                    fg = fgp.tile([128, CAP, ROW], BF16, tag="fg")
                    src_ap = tbl_lo if hf == 0 else tbl_hi
                    nc.gpsimd.dma_gather(
                        fg[:, 0:nt, :], src_ap,
                        idx_sb[:, colp:colp + nt * 8],
                        nt * 128, nt * 128, ROW, elem_step=ROW,
                        single_packet=False, queue_num=qn)
                    qn = (qn + 1) % 4
                    colp += nt * 8

                    t = 0
                    while t < nt:
                        g = min(4, nt - t)
                        # el[src] per edge: dot(feat, al) per head
                        eltmp = elp.tile([128, 4, 128], BF16, tag="eltmp")
                        al_b = (ALt[layer][:].unsqueeze(1)
                                .broadcast_to([128, g, 128]))
                        nc.vector.tensor_tensor(
                            eltmp[:, 0:g, :], fg[:, t:t + g, :], al_b,
                            op=OP.mult)
                        el = smp.tile([128, 4 * 4], F32, tag="el")
                        nc.vector.tensor_reduce(
                            el[:, 0:g * H],
                            eltmp[:, 0:g, :].rearrange(
                                "p g (h d) -> p (g h) d", h=H),
                            axis=AX.X, op=OP.add)
                        sx = smp.tile([128, 4, 4], F32, tag="sx")
                        ux = smp.tile([128, 4, 4], F32, tag="ux")
                        ex = smp.tile([128, 4, 4], F32, tag="exx")
                        exb = smp.tile([128, 4, 4], BF16, tag="exb")
                        er_b = (elerB[:, w, 4:4 + H].unsqueeze(1)
                                .broadcast_to([128, g, H]))
                        nc.vector.tensor_tensor(
                            sx[:, 0:g, 0:H],
                            el[:, 0:g * H].rearrange("p (g h) -> p g h", h=H),
                            er_b, op=OP.add)
                        nc.scalar.activation(ux[:, 0:g, 0:H], sx[:, 0:g, 0:H],
                                             AF.Exp, bias=negC[:, 0:1],
                                             scale=1.0)
                        nc.scalar.activation(ex[:, 0:g, 0:H], sx[:, 0:g, 0:H],
                                             AF.Exp, bias=negC[:, 0:1],
                                             scale=NEG_SLOPE)
                        val_b = (valid_sb[:, g0 + t:g0 + t + g].unsqueeze(2)
                                 .broadcast_to([128, g, H]))
                        nc.vector.scalar_tensor_tensor(
                            ex[:, 0:g, 0:H], ux[:, 0:g, 0:H], 1.0,
                            ex[:, 0:g, 0:H], op0=OP.mult, op1=OP.max)
                        nc.vector.tensor_tensor(exb[:, 0:g, 0:H],
                                                ex[:, 0:g, 0:H], val_b,
                                                op=OP.mult)
                        mext = mxp.tile([128, 4, 132], BF16, tag="mext")
                        ex_b = (exb[:, 0:g, 0:H].unsqueeze(3)
                                .broadcast_to([128, g, H, D]))
                        nc.vector.tensor_tensor(
                            mext[:, 0:g, 0:128]
                            .rearrange("p g (h d) -> p g h d", h=H),
                            fg[:, t:t + g, :]
                            .rearrange("p g (h d) -> p g h d", h=H),
                            ex_b, op=OP.mult)
                        nc.vector.tensor_copy(mext[:, 0:g, 128:128 + H],
                                              exb[:, 0:g, 0:H])
                        for k in range(g):
                            done_w[w] += 1
                            nc.tensor.matmul(
                                acc_ps[:, 0:128 + H], identb[:, :],
                                mext[:, k, 0:128 + H],
                                start=first_mm,
                                stop=(done_w[w] == ntiles_w[w]))
                            first_mm = False
                        t += g

                    if done_w[w] == ntiles_w[w]:
                        dn = smp.tile([128, 8], F32, tag="dn")
                        nc.vector.tensor_scalar(dn[:, 0:H],
                                                acc_ps[:, 128:128 + H],
                                                1e-9, None, op0=OP.add)
                        nc.vector.reciprocal(dn[:, 4:4 + H], dn[:, 0:H])
                        rec_b = (dn[:, 4:4 + H].unsqueeze(2)
                                 .broadcast_to([128, H, D]))
                        if layer < 2:
                            hsb = smp.tile([128, 128], BF16, tag="hsb")
                            nc.vector.tensor_tensor(
                                hsb[:].rearrange("p (h d) -> p h d", h=H),
                                acc_ps[:, 0:128]
                                .rearrange("p (h d) -> p h d", h=H),
                                rec_b, op=OP.mult)
                            hT_ps = psp.tile([128, 128], BF16, tag="psb")
                            nc.tensor.matmul(hT_ps[:], hsb[:], identb[:, :],
                                             is_transpose=True,
                                             start=True, stop=True)
                            nc.scalar.activation(hnext[:, w, :], hT_ps[:],
                                                 AF.Relu)
                        else:
                            hsb = smp.tile([128, 128], F32, tag="hsbf")
                            nc.vector.tensor_tensor(
                                hsb[:].rearrange("p (h d) -> p h d", h=H),
                                acc_ps[:, 0:128]
                                .rearrange("p (h d) -> p h d", h=H),
                                rec_b, op=OP.mult)
                            nc.sync.dma_start(
                                out_p[:, :].rearrange("(w p) f -> w p f",
                                                      p=128)[w, :, :],
                                hsb[:])
    nc.finalize()
    return nc


# ---------------------------------------------------------------------------
# Entry point
# ---------------------------------------------------------------------------

def kernel(features, src, dst, W0, al0, ar0, W1, al1, ar1, W2, al2, ar2):
    out, _ = run_gat(features, src, dst, W0, al0, ar0, W1, al1, ar1,
                     W2, al2, ar2, trace=False)
    return out


def run_gat(features, src, dst, W0, al0, ar0, W1, al1, ar1, W2, al2, ar2,
            trace=False):
    features = np.asarray(features, dtype=np.float32)
    n_nodes = features.shape[0]
    meta = preprocess(src, dst, n_nodes)
    NP, W, npc = meta["NP"], meta["W"], meta["npc"]

    Wm0, A0, AL0 = pack_weights(np.asarray(W0), al0, ar0)
    Wm1, A1, AL1 = pack_weights(np.asarray(W1), al1, ar1)
    Wm2, A2, AL2 = pack_weights(np.asarray(W2), al2, ar2)

    ident = np.eye(128, dtype=np.float32)
    identb = np.eye(128, dtype=np.float32).astype(BF)
    ones1 = np.ones((1, 128), dtype=np.float32)
    onescol = np.ones((128, 1), dtype=np.float32)

    in_maps = []
    for c in range(N_CORES):
        h_c = np.zeros((NP, 128), dtype=np.float32)
        h_c[:npc] = features[c * npc:(c + 1) * npc][meta["perm"][c]]
        in_maps.append({
            "hT0": np.ascontiguousarray(h_c.T).astype(BF),
            "idx": meta["idx_img"][c],
            "valid": meta["valid"][c],
            "W0": Wm0.astype(BF), "W1": Wm1.astype(BF), "W2": Wm2.astype(BF),
            "A0": A0.astype(BF), "A1": A1.astype(BF), "A2": A2.astype(BF),
            "AL0": AL0.astype(BF), "AL1": AL1.astype(BF),
            "AL2": AL2.astype(BF),
            "ident": ident, "identb": identb, "ones1": ones1,
            "onescol": onescol,
        })

    nc = build_nc(meta)
    br = run_bass_kernel_spmd(nc, in_maps, list(range(N_CORES)), trace=trace)
    res = br.results

    out = np.empty((n_nodes, 128), dtype=np.float32)
    for c in range(N_CORES):
        o = np.asarray(res[c]["out"])
        out[c * npc:(c + 1) * npc] = o[np.argsort(meta["perm"][c])]
    return out, br


# revision 12
# speedup vs baseline: 1.5721x; 1.3164x over previous
"""GAT (3-layer, DGL-style) on 8 Trainium2 NeuronCores.

Sharding: nodes across the 8 cores (6250 each, padded to 6272 = 49*128),
per-core nodes permuted by descending in-degree.  A "window" is 128 nodes;
a node is pinned to one SBUF partition lane of its window.  Per layer:

  Phase A (node side): featT = W^T @ h^T per window on PE (bf16), er via a
  small second matmul, write 256-byte bf16 feature rows to local DRAM,
  AllGather the 12.8 MB table across cores.

  Phase B (edge side): per window, edge tiles of 128 edges = one in-edge per
  destination partition.  dma_gather fetches 256B bf16 rows (int16 indices;
  the 50176-row table is indexed as two 25088-row halves).  el[src] is
  recomputed per edge on the vector engine (dot with the al vector
  replicated across partitions); er[dst] is a per-partition constant.
  exp(lrelu(s)-C) = max(exp(s-C), exp(0.2*s-C)) on ACT.  Messages (+
  per-head exp columns) are segment-summed by an identity-lhsT bf16 PE
  matmul accumulating into one PSUM bank per window.

C is a per-core bound lrelu(max el + max er) + 3 computed on device; shifting
exp by C instead of the per-segment max changes the reference's +1e-9 epsilon
term by < 1e-3 relative.
"""

import sys

sys.path.insert(0, "/opt/trn_rl_repo")

import ml_dtypes
import numpy as np

import concourse.bass as bass
import concourse.bacc as bacc
import concourse.mybir as mybir
import concourse.tile as tile
from concourse.bass_utils import run_bass_kernel_spmd

F32 = mybir.dt.float32
BF16 = mybir.dt.bfloat16
I16 = mybir.dt.int16
AF = mybir.ActivationFunctionType
OP = mybir.AluOpType
AX = mybir.AxisListType

N_CORES = 8
DIM = 128
ROW = 256              # bf16 elems per table row (512 B: feat 128 | el 4 | pad)
CAP = 16               # max tiles per dma_gather call
NEG_SLOPE = 0.2
C_MARGIN = 3.0
HEADS = (4, 4, 1)
BF = ml_dtypes.bfloat16


# ---------------------------------------------------------------------------
# Host-side preprocessing
# ---------------------------------------------------------------------------

def preprocess(src, dst, n_nodes):
    src = np.asarray(src).astype(np.int64)
    dst = np.asarray(dst).astype(np.int64)
    npc = n_nodes // N_CORES
    NP = ((npc + 127) // 128) * 128
    W = NP // 128
    HALF = 4 * NP
    assert HALF <= 32768, HALF

    core = dst // npc
    local = dst - core * npc

    perm = []
    pos_of = np.empty(n_nodes, dtype=np.int64)
    for c in range(N_CORES):
        deg_c = np.bincount(local[core == c], minlength=npc)
        p = np.argsort(-deg_c, kind="stable")
        perm.append(p)
        inv = np.empty(npc, dtype=np.int64)
        inv[p] = np.arange(npc)
        pos_of[c * npc:(c + 1) * npc] = inv
    row_of = (np.arange(n_nodes) // npc) * NP + pos_of

    seg_pos = pos_of[dst]
    wv = seg_pos // 128
    pv = seg_pos % 128
    half = (row_of[src] >= HALF).astype(np.int64)

    # occurrence rank within (core, seg, half)
    key = (core * NP + seg_pos) * 2 + half
    order = np.argsort(key, kind="stable")
    ks = key[order]
    starts = np.r_[0, np.flatnonzero(np.diff(ks)) + 1]
    gid = np.zeros(len(ks), dtype=np.int64)
    gid[starts[1:]] = 1
    gid = np.cumsum(gid)
    t_in = np.arange(len(ks)) - starts[gid]
    tv = np.empty(len(ks), dtype=np.int64)
    tv[order] = t_in

    cnt = np.bincount(key, minlength=N_CORES * NP * 2).reshape(
        N_CORES, W, 128, 2)
    T_lo = cnt[:, :, :, 0].max(axis=(0, 2)).astype(np.int64)
    T_hi = cnt[:, :, :, 1].max(axis=(0, 2)).astype(np.int64)

    calls = []
    for w in range(W):
        for hf, T in ((0, int(T_lo[w])), (1, int(T_hi[w]))):
            t = 0
            while t < T:
                nt = min(CAP, T - t)
                calls.append((w, hf, nt))
                t += nt
    gtot = int(T_lo.sum() + T_hi.sum())
    icols = 8 * sum(nt for (_, _, nt) in calls)

    tile_off = np.zeros((W, 2), dtype=np.int64)
    acc = 0
    for w in range(W):
        tile_off[w, 0] = acc
        acc += T_lo[w]
        tile_off[w, 1] = acc
        acc += T_hi[w]

    idx_imgs, valids = [], []
    for c in range(N_CORES):
        m = core == c
        slots_idx = np.zeros((128, gtot), dtype=np.int64)
        slots_val = np.zeros((128, gtot), dtype=np.float32)
        g = tile_off[wv[m], half[m]] + tv[m]
        slots_idx[pv[m], g] = row_of[src[m]] - half[m] * HALF
        slots_val[pv[m], g] = 1.0
        img = np.zeros((16, icols), dtype=np.int16)
        colp = 0
        tile_ptr = {}
        for (w, hf, nt) in calls:
            t0 = tile_ptr.get((w, hf), 0)
            g0 = tile_off[w, hf] + t0
            part = slots_idx[:, g0:g0 + nt]          # [128, nt]
            flat = part.T.reshape(-1)                # j = t*128 + p
            img[:, colp:colp + nt * 8] = flat.reshape(nt * 8, 16).T
            colp += nt * 8
            tile_ptr[(w, hf)] = t0 + nt
        idx_imgs.append(np.ascontiguousarray(np.tile(img, (8, 1))))
        valids.append(slots_val)

    return dict(perm=perm, calls=calls, T_lo=T_lo, T_hi=T_hi,
                idx_img=idx_imgs, valid=valids, NP=NP, W=W, gtot=gtot,
                icols=icols, npc=npc, HALF=HALF,
                tile_off=tile_off)


def pack_weights(Wl, al, ar):
    H, Dh = Wl.shape[1], Wl.shape[2]
    Wm = np.ascontiguousarray(np.asarray(Wl, dtype=np.float32)
                              .reshape(Wl.shape[0], H * Dh))
    A = np.zeros((H * Dh, 8), dtype=np.float32)
    for h in range(H):
        A[h * Dh:(h + 1) * Dh, h] = np.asarray(al, dtype=np.float32)[h]
        A[h * Dh:(h + 1) * Dh, 4 + h] = np.asarray(ar, dtype=np.float32)[h]
    return Wm, A


# ---------------------------------------------------------------------------
# Device kernel
# ---------------------------------------------------------------------------

def build_nc(meta):
    NP, W, gtot, icols = meta["NP"], meta["W"], meta["gtot"], meta["icols"]
    calls, HALF = meta["calls"], meta["HALF"]
    NTOT = N_CORES * NP
    tile_off = meta["tile_off"]

    nc = bacc.Bacc(None, target_bir_lowering=False, debug=False,
                   num_devices=N_CORES, num_swdge_queues=4)

    hT0 = nc.declare_dram_parameter("hT0", [128, NP], BF16, isOutput=False)
    idx_p = nc.declare_dram_parameter("idx", [128, icols], I16, isOutput=False)
    val_p = nc.declare_dram_parameter("valid", [128, gtot], F32,
                                      isOutput=False)
    Wp = [nc.declare_dram_parameter(f"W{l}", [128, 128], BF16, isOutput=False)
          for l in range(3)]
    Ap = [nc.declare_dram_parameter(f"A{l}", [128, 8], BF16, isOutput=False)
          for l in range(3)]
    ident_p = nc.declare_dram_parameter("ident", [128, 128], F32,
                                        isOutput=False)
    identb_p = nc.declare_dram_parameter("identb", [128, 128], BF16,
                                         isOutput=False)
    ones_p = nc.declare_dram_parameter("ones1", [1, 128], F32, isOutput=False)
    onescol_p = nc.declare_dram_parameter("onescol", [128, 1], F32,
                                          isOutput=False)
    out_p = nc.declare_dram_parameter("out", [NP, 128], F32, isOutput=True)

    with tile.TileContext(nc) as tc:
        with (
            tc.tile_pool(name="const", bufs=1) as constp,
            tc.tile_pool(name="persist", bufs=1) as pers,
            tc.tile_pool(name="featg", bufs=6) as fgp,
            tc.tile_pool(name="mext", bufs=4) as mxp,
            tc.tile_pool(name="small", bufs=4) as smp,
            tc.tile_pool(name="psum", bufs=3, space="PSUM") as psp,
            tc.tile_pool(name="psacc", bufs=2, space="PSUM") as psaccp,
            tc.tile_pool(name="dram", bufs=1, space="DRAM") as dramp,
        ):
            ident = constp.tile([128, 128], F32, tag="ident")
            nc.sync.dma_start(ident[:], ident_p[:, :])
            identb = constp.tile([128, 128], BF16, tag="identb")
            nc.sync.dma_start(identb[:], identb_p[:, :])
            ones1 = constp.tile([1, 128], F32, tag="ones1")
            nc.sync.dma_start(ones1[:], ones_p[:, :])
            onescol = constp.tile([128, 1], F32, tag="onescol")
            nc.sync.dma_start(onescol[:], onescol_p[:, :])
            Wt = [constp.tile([128, 128], BF16, tag=f"W{l}", name=f"Wt{l}")
                  for l in range(3)]
            At = [constp.tile([128, 8], BF16, tag=f"A{l}", name=f"At{l}")
                  for l in range(3)]
            for l in range(3):
                nc.sync.dma_start(Wt[l][:], Wp[l][:, :])
                nc.sync.dma_start(At[l][:], Ap[l][:, :])
            idx_sb = pers.tile([128, icols], I16, tag="idx")
            nc.sync.dma_start(idx_sb[:], idx_p[:, :])
            valid_sb = pers.tile([128, gtot], F32, tag="valid")
            nc.sync.dma_start(valid_sb[:], val_p[:, :])

            hT = [pers.tile([128, W, 128], BF16, tag=f"hT{i}", name=f"hT{i}")
                  for i in range(2)]
            nc.sync.dma_start(hT[0][:, :, :],
                              hT0[:, :].rearrange("p (w n) -> p w n", w=W))

            elerB = pers.tile([128, W, 8], F32, tag="elerB")
            rowimg = pers.tile([128, W, ROW], BF16, tag="rowimg")

            loc_tbl = dramp.tile([NP, ROW], BF16, tag="loctbl")
            full_tbl = dramp.tile([NTOT, ROW], BF16, tag="fulltbl")

            for layer in range(3):
                H = HEADS[layer]
                D = 128 // H
                hcur, hnext = hT[layer % 2], hT[(layer + 1) % 2]

                # ======== Phase A ========
                for w in range(W):
                    featT_ps = psp.tile([128, 128], F32, tag="ps")
                    nc.tensor.matmul(featT_ps[:], Wt[layer][:],
                                     hcur[:, w, :], start=True, stop=True)
                    featT_sb = smp.tile([128, 128], BF16, tag="featT_sb")
                    nc.vector.tensor_copy(featT_sb[:], featT_ps[:])
                    elerT_ps = psp.tile([8, 128], F32, tag="ps")
                    nc.tensor.matmul(elerT_ps[:], At[layer][:], featT_sb[:],
                                     start=True, stop=True)
                    elerT_sb = smp.tile([8, 128], F32, tag="elerT_sb")
                    nc.vector.tensor_copy(elerT_sb[:], elerT_ps[:])
                    eler_ps = psp.tile([128, 8], F32, tag="ps")
                    nc.tensor.matmul(eler_ps[:], elerT_sb[:],
                                     ident[0:8, 0:8], is_transpose=True,
                                     start=True, stop=True)
                    nc.vector.tensor_copy(elerB[:, w, :], eler_ps[:])
                    feat_ps = psp.tile([128, 128], BF16, tag="psb")
                    nc.tensor.matmul(feat_ps[:], featT_sb[:], identb[:, :],
                                     is_transpose=True, start=True, stop=True)
                    nc.vector.tensor_copy(rowimg[:, w, 0:128], feat_ps[:])
                    nc.vector.tensor_copy(rowimg[:, w, 128:128 + H],
                                          eler_ps[:, 0:H])
                    nc.sync.dma_start(
                        loc_tbl[:].rearrange("(w p) f -> w p f", p=128)
                        [w, :, 0:128 + H],
                        rowimg[:, w, 0:128 + H])

                # ---- AllGather ----
                nc.gpsimd.collective_compute(
                    "AllGather", OP.bypass,
                    replica_groups=[list(range(N_CORES))],
                    ins=[loc_tbl[:].opt()], outs=[full_tbl[:].opt()])

                # ---- -C = -(lrelu(max el + max er) + margin) ----
                mx = smp.tile([128, 2], F32, tag="mx")
                nc.vector.tensor_reduce(mx[:, 0:1], elerB[:, :, 0:H],
                                        axis=AX.XY, op=OP.max)
                nc.vector.tensor_reduce(mx[:, 1:2], elerB[:, :, 4:4 + H],
                                        axis=AX.XY, op=OP.max)
                mxT_ps = psp.tile([2, 128], F32, tag="ps")
                nc.tensor.matmul(mxT_ps[:], mx[:], ident[:, :],
                                 is_transpose=True, start=True, stop=True)
                mm = smp.tile([2, 1], F32, tag="mm")
                nc.vector.tensor_reduce(mm[:], mxT_ps[:, :], axis=AX.X,
                                        op=OP.max)
                s_ps = psp.tile([1, 1], F32, tag="ps")
                nc.tensor.matmul(s_ps[:], mm[:], onescol[0:2, 0:1],
                                 start=True, stop=True)
                cs = smp.tile([1, 4], F32, tag="cs")
                nc.vector.tensor_copy(cs[:, 0:1], s_ps[:])
                nc.vector.tensor_scalar(cs[:, 1:2], cs[:, 0:1], NEG_SLOPE,
                                        None, op0=OP.mult)
                nc.vector.tensor_tensor(cs[:, 2:3], cs[:, 0:1],
                                        cs[:, 1:2], op=OP.max)
                nc.vector.tensor_scalar(cs[:, 3:4], cs[:, 2:3], -1.0,
                                        -C_MARGIN, op0=OP.mult,
                                        op1=OP.add)
                negC_ps = psp.tile([128, 1], F32, tag="ps")
                nc.tensor.matmul(negC_ps[:], ones1[:], cs[:, 3:4],
                                 start=True, stop=True)
                negC = smp.tile([128, 1], F32, tag="negC")
                nc.vector.tensor_copy(negC[:], negC_ps[:])

                # ======== Phase B ========
                tbl_lo = full_tbl[0:HALF, :]
                tbl_hi = full_tbl[HALF:NTOT, :]
                colp = 0
                tile_ptr = {}
                cur_w = -1
                acc_ps = None
                first_mm = True
                ntiles_w = {w: int(meta["T_lo"][w] + meta["T_hi"][w])
                            for w in range(W)}
                done_w = {w: 0 for w in range(W)}
                qn = 0
                for (w, hf, nt) in calls:
                    if w != cur_w:
                        cur_w = w
                        acc_ps = psaccp.tile([128, 132], F32, tag="acc")
                        first_mm = True
                    t0 = tile_ptr.get((w, hf), 0)
                    tile_ptr[(w, hf)] = t0 + nt
                    g0 = int(tile_off[w, hf]) + t0

                    fg = fgp.tile([128, CAP, ROW], BF16, tag="fg")
                    src_ap = tbl_lo if hf == 0 else tbl_hi
                    nc.gpsimd.dma_gather(
                        fg[:, 0:nt, :], src_ap,
                        idx_sb[:, colp:colp + nt * 8],
                        nt * 128, nt * 128, ROW, elem_step=ROW,
                        single_packet=False, queue_num=qn)
                    qn = (qn + 1) % 4
                    colp += nt * 8

                    t = 0
                    while t < nt:
                        g = min(4, nt - t)
                        sx = smp.tile([128, 4, 4], F32, tag="sx")
                        ux = smp.tile([128, 4, 4], F32, tag="ux")
                        ex = smp.tile([128, 4, 4], F32, tag="exx")
                        exb = smp.tile([128, 4, 4], BF16, tag="exb")
                        er_b = (elerB[:, w, 4:4 + H].unsqueeze(1)
                                .broadcast_to([128, g, H]))
                        nc.vector.tensor_tensor(
                            sx[:, 0:g, 0:H], fg[:, t:t + g, 128:128 + H],
                            er_b, op=OP.add)
                        nc.scalar.activation(ux[:, 0:g, 0:H], sx[:, 0:g, 0:H],
                                             AF.Exp, bias=negC[:, 0:1],
                                             scale=1.0)
                        nc.scalar.activation(ex[:, 0:g, 0:H], sx[:, 0:g, 0:H],
                                             AF.Exp, bias=negC[:, 0:1],
                                             scale=NEG_SLOPE)
                        val_b = (valid_sb[:, g0 + t:g0 + t + g].unsqueeze(2)
                                 .broadcast_to([128, g, H]))
                        nc.vector.scalar_tensor_tensor(
                            ex[:, 0:g, 0:H], ux[:, 0:g, 0:H], 1.0,
                            ex[:, 0:g, 0:H], op0=OP.mult, op1=OP.max)
                        nc.vector.tensor_tensor(exb[:, 0:g, 0:H],
                                                ex[:, 0:g, 0:H], val_b,
                                                op=OP.mult)
                        mext = mxp.tile([128, 4, 132], BF16, tag="mext")
                        ex_b = (exb[:, 0:g, 0:H].unsqueeze(3)
                                .broadcast_to([128, g, H, D]))
                        nc.vector.tensor_tensor(
                            mext[:, 0:g, 0:128]
                            .rearrange("p g (h d) -> p g h d", h=H),
                            fg[:, t:t + g, 0:128]
                            .rearrange("p g (h d) -> p g h d", h=H),
                            ex_b, op=OP.mult)
                        nc.vector.tensor_copy(mext[:, 0:g, 128:128 + H],
                                              exb[:, 0:g, 0:H])
                        for k in range(g):
                            done_w[w] += 1
                            nc.tensor.matmul(
                                acc_ps[:, 0:128 + H], identb[:, :],
                                mext[:, k, 0:128 + H],
                                start=first_mm,
                                stop=(done_w[w] == ntiles_w[w]))
                            first_mm = False
                        t += g

                    if done_w[w] == ntiles_w[w]:
                        dn = smp.tile([128, 8], F32, tag="dn")
                        nc.vector.tensor_scalar(dn[:, 0:H],
                                                acc_ps[:, 128:128 + H],
                                                1e-9, None, op0=OP.add)
                        nc.vector.reciprocal(dn[:, 4:4 + H], dn[:, 0:H])
                        rec_b = (dn[:, 4:4 + H].unsqueeze(2)
                                 .broadcast_to([128, H, D]))
                        if layer < 2:
                            hsb = smp.tile([128, 128], BF16, tag="hsb")
                            nc.vector.tensor_tensor(
                                hsb[:].rearrange("p (h d) -> p h d", h=H),
                                acc_ps[:, 0:128]
                                .rearrange("p (h d) -> p h d", h=H),
                                rec_b, op=OP.mult)
                            hT_ps = psp.tile([128, 128], BF16, tag="psb")
                            nc.tensor.matmul(hT_ps[:], hsb[:], identb[:, :],
                                             is_transpose=True,
                                             start=True, stop=True)
                            nc.scalar.activation(hnext[:, w, :], hT_ps[:],
                                                 AF.Relu)
                        else:
                            hsb = smp.tile([128, 128], F32, tag="hsbf")
                            nc.vector.tensor_tensor(
                                hsb[:].rearrange("p (h d) -> p h d", h=H),
                                acc_ps[:, 0:128]
                                .rearrange("p (h d) -> p h d", h=H),
                                rec_b, op=OP.mult)
                            nc.sync.dma_start(
                                out_p[:, :].rearrange("(w p) f -> w p f",
                                                      p=128)[w, :, :],
                                hsb[:])
    nc.finalize()
    return nc


# ---------------------------------------------------------------------------
# Entry point
# ---------------------------------------------------------------------------

def kernel(features, src, dst, W0, al0, ar0, W1, al1, ar1, W2, al2, ar2):
    out, _ = run_gat(features, src, dst, W0, al0, ar0, W1, al1, ar1,
                     W2, al2, ar2, trace=False)
    return out


def run_gat(features, src, dst, W0, al0, ar0, W1, al1, ar1, W2, al2, ar2,
            trace=False):
    features = np.asarray(features, dtype=np.float32)
    n_nodes = features.shape[0]
    meta = preprocess(src, dst, n_nodes)
    NP, W, npc = meta["NP"], meta["W"], meta["npc"]

    Wm0, A0 = pack_weights(np.asarray(W0), al0, ar0)
    Wm1, A1 = pack_weights(np.asarray(W1), al1, ar1)
    Wm2, A2 = pack_weights(np.asarray(W2), al2, ar2)

    ident = np.eye(128, dtype=np.float32)
    identb = np.eye(128, dtype=np.float32).astype(BF)
    ones1 = np.ones((1, 128), dtype=np.float32)
    onescol = np.ones((128, 1), dtype=np.float32)

    in_maps = []
    for c in range(N_CORES):
        h_c = np.zeros((NP, 128), dtype=np.float32)
        h_c[:npc] = features[c * npc:(c + 1) * npc][meta["perm"][c]]
        in_maps.append({
            "hT0": np.ascontiguousarray(h_c.T).astype(BF),
            "idx": meta["idx_img"][c],
            "valid": meta["valid"][c],
            "W0": Wm0.astype(BF), "W1": Wm1.astype(BF), "W2": Wm2.astype(BF),
            "A0": A0.astype(BF), "A1": A1.astype(BF), "A2": A2.astype(BF),
            "ident": ident, "identb": identb, "ones1": ones1,
            "onescol": onescol,
        })

    nc = build_nc(meta)
    br = run_bass_kernel_spmd(nc, in_maps, list(range(N_CORES)), trace=trace)
    res = br.results

    out = np.empty((n_nodes, 128), dtype=np.float32)
    for c in range(N_CORES):
        o = np.asarray(res[c]["out"])
        out[c * npc:(c + 1) * npc] = o[np.argsort(meta["perm"][c])]
    return out, br


# revision 23
# speedup vs baseline: 1.6847x; 1.0716x over previous
"""GAT (3-layer, DGL-style) on 8 Trainium2 NeuronCores.

Sharding: nodes across the 8 cores (6250 each, padded to 6272 = 49*128),
per-core nodes permuted by descending in-degree.  A "window" is 128 nodes;
a node is pinned to one SBUF partition lane of its window.  Per layer:

  Phase A (node side): featT = W^T @ h^T per window on PE (bf16), er via a
  small second matmul, write 256-byte bf16 feature rows to local DRAM,
  AllGather the 12.8 MB table across cores.

  Phase B (edge side): per window, edge tiles of 128 edges = one in-edge per
  destination partition.  dma_gather fetches 256B bf16 rows (int16 indices;
  the 50176-row table is indexed as two 25088-row halves).  el[src] is
  recomputed per edge on the vector engine (dot with the al vector
  replicated across partitions); er[dst] is a per-partition constant.
  exp(lrelu(s)-C) = max(exp(s-C), exp(0.2*s-C)) on ACT.  Messages (+
  per-head exp columns) are segment-summed by an identity-lhsT bf16 PE
  matmul accumulating into one PSUM bank per window.

C is a per-core bound lrelu(max el + max er) + 3 computed on device; shifting
exp by C instead of the per-segment max changes the reference's +1e-9 epsilon
term by < 1e-3 relative.
"""

import sys

sys.path.insert(0, "/opt/trn_rl_repo")

import ml_dtypes
import numpy as np

import concourse.bass as bass
import concourse.bacc as bacc
import concourse.mybir as mybir
import concourse.tile as tile
from concourse.bass_utils import run_bass_kernel_spmd

F32 = mybir.dt.float32
BF16 = mybir.dt.bfloat16
I16 = mybir.dt.int16
AF = mybir.ActivationFunctionType
OP = mybir.AluOpType
AX = mybir.AxisListType

N_CORES = 8
DIM = 128
ROW = 128              # bf16 elems per table row (256 B, y = feat @ M basis)
CAP = 16               # max tiles per dma_gather call
NEG_SLOPE = 0.2
C_MARGIN = 3.0
HEADS = (4, 4, 1)
BF = ml_dtypes.bfloat16


# ---------------------------------------------------------------------------
# Host-side preprocessing
# ---------------------------------------------------------------------------

def preprocess(src, dst, n_nodes):
    src = np.asarray(src).astype(np.int64)
    dst = np.asarray(dst).astype(np.int64)
    npc = n_nodes // N_CORES
    NP = ((npc + 127) // 128) * 128
    W = NP // 128
    HALF = 4 * NP
    assert HALF <= 32768, HALF

    core = dst // npc
    local = dst - core * npc

    perm = []
    pos_of = np.empty(n_nodes, dtype=np.int64)
    for c in range(N_CORES):
        deg_c = np.bincount(local[core == c], minlength=npc)
        p = np.argsort(-deg_c, kind="stable")
        perm.append(p)
        inv = np.empty(npc, dtype=np.int64)
        inv[p] = np.arange(npc)
        pos_of[c * npc:(c + 1) * npc] = inv
    row_of = (np.arange(n_nodes) // npc) * NP + pos_of

    seg_pos = pos_of[dst]
    wv = seg_pos // 128
    pv = seg_pos % 128
    half = (row_of[src] >= HALF).astype(np.int64)

    # occurrence rank within (core, seg, half)
    key = (core * NP + seg_pos) * 2 + half
    order = np.argsort(key, kind="stable")
    ks = key[order]
    starts = np.r_[0, np.flatnonzero(np.diff(ks)) + 1]
    gid = np.zeros(len(ks), dtype=np.int64)
    gid[starts[1:]] = 1
    gid = np.cumsum(gid)
    t_in = np.arange(len(ks)) - starts[gid]
    tv = np.empty(len(ks), dtype=np.int64)
    tv[order] = t_in

    cnt = np.bincount(key, minlength=N_CORES * NP * 2).reshape(
        N_CORES, W, 128, 2)
    T_lo = cnt[:, :, :, 0].max(axis=(0, 2)).astype(np.int64)
    T_hi = cnt[:, :, :, 1].max(axis=(0, 2)).astype(np.int64)

    calls = []
    for w in range(W):
        for hf, T in ((0, int(T_lo[w])), (1, int(T_hi[w]))):
            t = 0
            while t < T:
                nt = min(CAP, T - t)
                calls.append((w, hf, nt))
                t += nt
    gtot = int(T_lo.sum() + T_hi.sum())
    icols = 8 * sum(nt for (_, _, nt) in calls)

    tile_off = np.zeros((W, 2), dtype=np.int64)
    acc = 0
    for w in range(W):
        tile_off[w, 0] = acc
        acc += T_lo[w]
        tile_off[w, 1] = acc
        acc += T_hi[w]

    idx_imgs, valids = [], []
    for c in range(N_CORES):
        m = core == c
        slots_idx = np.zeros((128, gtot), dtype=np.int64)
        slots_val = np.zeros((128, gtot), dtype=np.float32)
        g = tile_off[wv[m], half[m]] + tv[m]
        slots_idx[pv[m], g] = row_of[src[m]] - half[m] * HALF
        slots_val[pv[m], g] = 1.0
        img = np.zeros((16, icols), dtype=np.int16)
        colp = 0
        tile_ptr = {}
        for (w, hf, nt) in calls:
            t0 = tile_ptr.get((w, hf), 0)
            g0 = tile_off[w, hf] + t0
            part = slots_idx[:, g0:g0 + nt]          # [128, nt]
            flat = part.T.reshape(-1)                # j = t*128 + p
            img[:, colp:colp + nt * 8] = flat.reshape(nt * 8, 16).T
            colp += nt * 8
            tile_ptr[(w, hf)] = t0 + nt
        idx_imgs.append(np.ascontiguousarray(np.tile(img, (8, 1))))
        valids.append(slots_val)

    return dict(perm=perm, calls=calls, T_lo=T_lo, T_hi=T_hi,
                idx_img=idx_imgs, valid=valids, NP=NP, W=W, gtot=gtot,
                icols=icols, npc=npc, HALF=HALF,
                tile_off=tile_off)


def pack_weights(Wl, al, ar):
    """Head-block basis change: y = feat @ M where M's block h has s_h*al_h
    as column 0 (so el_h = y[32h] * al_h[j_h]) and unit columns elsewhere.
    Returns W@M, Minv@A (for er + the C bound), Minv (posthoc undo), and
    crep (the per-head el rescale, replicated to 128 partitions)."""
    H, Dh = Wl.shape[1], Wl.shape[2]
    Wm = np.asarray(Wl, dtype=np.float64).reshape(Wl.shape[0], H * Dh)
    al = np.asarray(al, dtype=np.float64)
    ar = np.asarray(ar, dtype=np.float64)
    M = np.zeros((H * Dh, H * Dh))
    c = np.zeros(H)
    for h in range(H):
        blk = slice(h * Dh, (h + 1) * Dh)
        jh = int(np.argmax(np.abs(al[h])))
        c[h] = al[h][jh]
        Mh = np.zeros((Dh, Dh))
        Mh[:, 0] = al[h] / c[h]
        k = 1
        for j in range(Dh):
            if j != jh:
                Mh[j, k] = 1.0
                k += 1
        M[blk, blk] = Mh
    Minv = np.linalg.inv(M)
    WmM = np.ascontiguousarray((Wm @ M).astype(np.float32))
    A = np.zeros((H * Dh, 8))
    for h in range(H):
        A[h * Dh:(h + 1) * Dh, h] = al[h]
        A[h * Dh:(h + 1) * Dh, 4 + h] = ar[h]
    Ap = np.ascontiguousarray((Minv @ A).astype(np.float32))
    c4 = np.zeros(4, dtype=np.float32)
    c4[:H] = c
    crep = np.ascontiguousarray(np.tile(c4[None, :], (128, 1)))
    Minv128 = np.zeros((128, 128), dtype=np.float32)
    Minv128[:H * Dh, :H * Dh] = Minv
    return WmM, Ap, np.ascontiguousarray(Minv128), crep


# ---------------------------------------------------------------------------
# Device kernel
# ---------------------------------------------------------------------------

def build_nc(meta):
    NP, W, gtot, icols = meta["NP"], meta["W"], meta["gtot"], meta["icols"]
    calls, HALF = meta["calls"], meta["HALF"]
    NTOT = N_CORES * NP
    tile_off = meta["tile_off"]

    nc = bacc.Bacc(None, target_bir_lowering=False, debug=False,
                   num_devices=N_CORES, num_swdge_queues=4)

    hT0 = nc.declare_dram_parameter("hT0", [128, NP], BF16, isOutput=False)
    idx_p = nc.declare_dram_parameter("idx", [128, icols], I16, isOutput=False)
    val_p = nc.declare_dram_parameter("valid", [128, gtot], F32,
                                      isOutput=False)
    Wp = [nc.declare_dram_parameter(f"W{l}", [128, 128], BF16, isOutput=False)
          for l in range(3)]
    Ap = [nc.declare_dram_parameter(f"A{l}", [128, 8], BF16, isOutput=False)
          for l in range(3)]
    Mp = [nc.declare_dram_parameter(f"Minv{l}", [128, 128], BF16,
                                    isOutput=False) for l in range(3)]
    Cp = [nc.declare_dram_parameter(f"crep{l}", [128, 4], F32,
                                    isOutput=False) for l in range(3)]
    ident_p = nc.declare_dram_parameter("ident", [128, 128], F32,
                                        isOutput=False)
    identb_p = nc.declare_dram_parameter("identb", [128, 128], BF16,
                                         isOutput=False)
    ones_p = nc.declare_dram_parameter("ones1", [1, 128], F32, isOutput=False)
    onescol_p = nc.declare_dram_parameter("onescol", [128, 1], F32,
                                          isOutput=False)
    out_p = nc.declare_dram_parameter("out", [NP, 128], F32, isOutput=True)

    with tile.TileContext(nc) as tc:
        with (
            tc.tile_pool(name="const", bufs=1) as constp,
            tc.tile_pool(name="persist", bufs=1) as pers,
            tc.tile_pool(name="featg", bufs=6) as fgp,
            tc.tile_pool(name="mext", bufs=4) as mxp,
            tc.tile_pool(name="small", bufs=4) as smp,
            tc.tile_pool(name="psum", bufs=3, space="PSUM") as psp,
            tc.tile_pool(name="psacc", bufs=2, space="PSUM") as psaccp,
            tc.tile_pool(name="dram", bufs=1, space="DRAM") as dramp,
        ):
            ident = constp.tile([128, 128], F32, tag="ident")
            nc.sync.dma_start(ident[:], ident_p[:, :])
            identb = constp.tile([128, 128], BF16, tag="identb")
            nc.sync.dma_start(identb[:], identb_p[:, :])
            ones1 = constp.tile([1, 128], F32, tag="ones1")
            nc.sync.dma_start(ones1[:], ones_p[:, :])
            onescol = constp.tile([128, 1], F32, tag="onescol")
            nc.sync.dma_start(onescol[:], onescol_p[:, :])
            Wt = [constp.tile([128, 128], BF16, tag=f"W{l}", name=f"Wt{l}")
                  for l in range(3)]
            At = [constp.tile([128, 8], BF16, tag=f"A{l}", name=f"At{l}")
                  for l in range(3)]
            Mt = [constp.tile([128, 128], BF16, tag=f"Minv{l}",
                              name=f"Mt{l}") for l in range(3)]
            Ct = [constp.tile([128, 4], F32, tag=f"crep{l}", name=f"Ct{l}")
                  for l in range(3)]
            for l in range(3):
                nc.sync.dma_start(Wt[l][:], Wp[l][:, :])
                nc.sync.dma_start(At[l][:], Ap[l][:, :])
                nc.sync.dma_start(Mt[l][:], Mp[l][:, :])
                nc.sync.dma_start(Ct[l][:], Cp[l][:, :])
            idx_sb = pers.tile([128, icols], I16, tag="idx")
            nc.sync.dma_start(idx_sb[:], idx_p[:, :])
            valid_sb = pers.tile([128, gtot], F32, tag="valid")
            nc.sync.dma_start(valid_sb[:], val_p[:, :])

            hT = [pers.tile([128, W, 128], BF16, tag=f"hT{i}", name=f"hT{i}")
                  for i in range(2)]
            nc.sync.dma_start(hT[0][:, :, :],
                              hT0[:, :].rearrange("p (w n) -> p w n", w=W))

            elerB = pers.tile([128, W, 8], F32, tag="elerB")
            rowimg = pers.tile([128, W, ROW], BF16, tag="rowimg")

            loc_tbl = dramp.tile([NP, ROW], BF16, tag="loctbl")
            full_tbl = dramp.tile([NTOT, ROW], BF16, tag="fulltbl")

            for layer in range(3):
                H = HEADS[layer]
                D = 128 // H
                hcur, hnext = hT[layer % 2], hT[(layer + 1) % 2]

                # ======== Phase A ========
                for w in range(W):
                    featT_ps = psp.tile([128, 128], F32, tag="ps")
                    nc.tensor.matmul(featT_ps[:], Wt[layer][:],
                                     hcur[:, w, :], start=True, stop=True)
                    featT_sb = smp.tile([128, 128], BF16, tag="featT_sb")
                    nc.vector.tensor_copy(featT_sb[:], featT_ps[:])
                    elerT_ps = psp.tile([8, 128], F32, tag="ps")
                    nc.tensor.matmul(elerT_ps[:], At[layer][:], featT_sb[:],
                                     start=True, stop=True)
                    elerT_sb = smp.tile([8, 128], F32, tag="elerT_sb")
                    nc.vector.tensor_copy(elerT_sb[:], elerT_ps[:])
                    eler_ps = psp.tile([128, 8], F32, tag="ps")
                    nc.tensor.matmul(eler_ps[:], elerT_sb[:],
                                     ident[0:8, 0:8], is_transpose=True,
                                     start=True, stop=True)
                    nc.vector.tensor_copy(elerB[:, w, :], eler_ps[:])
                    feat_ps = psp.tile([128, 128], BF16, tag="psb")
                    nc.tensor.matmul(feat_ps[:], featT_sb[:], identb[:, :],
                                     is_transpose=True, start=True, stop=True)
                    nc.vector.tensor_copy(rowimg[:, w, :], feat_ps[:])
                    nc.sync.dma_start(
                        loc_tbl[:].rearrange("(w p) f -> w p f", p=128)
                        [w, :, :],
                        rowimg[:, w, :])

                # ---- AllGather ----
                nc.gpsimd.collective_compute(
                    "AllGather", OP.bypass,
                    replica_groups=[list(range(N_CORES))],
                    ins=[loc_tbl[:].opt()], outs=[full_tbl[:].opt()])

                # ---- -C = -(lrelu(max el + max er) + margin) ----
                mx = smp.tile([128, 2], F32, tag="mx")
                nc.vector.tensor_reduce(mx[:, 0:1], elerB[:, :, 0:H],
                                        axis=AX.XY, op=OP.max)
                nc.vector.tensor_reduce(mx[:, 1:2], elerB[:, :, 4:4 + H],
                                        axis=AX.XY, op=OP.max)
                mxT_ps = psp.tile([2, 128], F32, tag="ps")
                nc.tensor.matmul(mxT_ps[:], mx[:], ident[:, :],
                                 is_transpose=True, start=True, stop=True)
                mm = smp.tile([2, 1], F32, tag="mm")
                nc.vector.tensor_reduce(mm[:], mxT_ps[:, :], axis=AX.X,
                                        op=OP.max)
                s_ps = psp.tile([1, 1], F32, tag="ps")
                nc.tensor.matmul(s_ps[:], mm[:], onescol[0:2, 0:1],
                                 start=True, stop=True)
                cs = smp.tile([1, 4], F32, tag="cs")
                nc.vector.tensor_copy(cs[:, 0:1], s_ps[:])
                nc.vector.tensor_scalar(cs[:, 1:2], cs[:, 0:1], NEG_SLOPE,
                                        None, op0=OP.mult)
                nc.vector.tensor_tensor(cs[:, 2:3], cs[:, 0:1],
                                        cs[:, 1:2], op=OP.max)
                nc.vector.tensor_scalar(cs[:, 3:4], cs[:, 2:3], -1.0,
                                        -C_MARGIN, op0=OP.mult,
                                        op1=OP.add)
                negC_ps = psp.tile([128, 1], F32, tag="ps")
                nc.tensor.matmul(negC_ps[:], ones1[:], cs[:, 3:4],
                                 start=True, stop=True)
                negC = smp.tile([128, 1], F32, tag="negC")
                nc.vector.tensor_copy(negC[:], negC_ps[:])

                # ======== Phase B ========
                tbl_lo = full_tbl[0:HALF, :]
                tbl_hi = full_tbl[HALF:NTOT, :]
                colp = 0
                tile_ptr = {}
                cur_w = -1
                acc_ps = None
                first_mm = True
                ntiles_w = {w: int(meta["T_lo"][w] + meta["T_hi"][w])
                            for w in range(W)}
                done_w = {w: 0 for w in range(W)}
                qn = 0
                for (w, hf, nt) in calls:
                    if w != cur_w:
                        cur_w = w
                        acc_ps = psaccp.tile([128, 132], F32, tag="acc")
                        first_mm = True
                    t0 = tile_ptr.get((w, hf), 0)
                    tile_ptr[(w, hf)] = t0 + nt
                    g0 = int(tile_off[w, hf]) + t0

                    fg = fgp.tile([128, CAP, ROW], BF16, tag="fg")
                    src_ap = tbl_lo if hf == 0 else tbl_hi
                    nc.gpsimd.dma_gather(
                        fg[:, 0:nt, :], src_ap,
                        idx_sb[:, colp:colp + nt * 8],
                        nt * 128, nt * 128, ROW, elem_step=ROW,
                        single_packet=False, queue_num=qn)
                    qn = (qn + 1) % 4
                    colp += nt * 8

                    t = 0
                    while t < nt:
                        g = min(4, nt - t)
                        sx = smp.tile([128, 4, 4], F32, tag="sx")
                        ux = smp.tile([128, 4, 4], F32, tag="ux")
                        ex = smp.tile([128, 4, 4], F32, tag="exx")
                        elg = smp.tile([128, 4, 4], F32, tag="elg")
                        c_b = (Ct[layer][:, 0:H].unsqueeze(1)
                               .broadcast_to([128, g, H]).unsqueeze(3))
                        nc.vector.tensor_tensor(
                            elg[:, 0:g, 0:H].unsqueeze(3),
                            fg[:, t:t + g, :].rearrange(
                                "p g (h d) -> p g h d", h=H)[:, :, :, 0:1],
                            c_b, op=OP.mult)
                        er_b = (elerB[:, w, 4:4 + H].unsqueeze(1)
                                .broadcast_to([128, g, H]))
                        nc.vector.tensor_tensor(
                            sx[:, 0:g, 0:H], elg[:, 0:g, 0:H],
                            er_b, op=OP.add)
                        nc.scalar.activation(ux[:, 0:g, 0:H], sx[:, 0:g, 0:H],
                                             AF.Exp, bias=negC[:, 0:1],
                                             scale=1.0)
                        nc.scalar.activation(ex[:, 0:g, 0:H], sx[:, 0:g, 0:H],
                                             AF.Exp, bias=negC[:, 0:1],
                                             scale=NEG_SLOPE)
                        val_b = (valid_sb[:, g0 + t:g0 + t + g].unsqueeze(2)
                                 .broadcast_to([128, g, H]))
                        nc.vector.scalar_tensor_tensor(
                            ex[:, 0:g, 0:H], ux[:, 0:g, 0:H], 1.0,
                            ex[:, 0:g, 0:H], op0=OP.mult, op1=OP.max)
                        mext = mxp.tile([128, 4, 132], BF16, tag="mext")
                        nc.vector.tensor_tensor(mext[:, 0:g, 128:128 + H],
                                                ex[:, 0:g, 0:H], val_b,
                                                op=OP.mult)
                        ex_b = (mext[:, 0:g, 128:128 + H].unsqueeze(3)
                                .broadcast_to([128, g, H, D]))
                        nc.vector.tensor_tensor(
                            mext[:, 0:g, 0:128]
                            .rearrange("p g (h d) -> p g h d", h=H),
                            fg[:, t:t + g, 0:128]
                            .rearrange("p g (h d) -> p g h d", h=H),
                            ex_b, op=OP.mult)
                        for k in range(g):
                            done_w[w] += 1
                            nc.tensor.matmul(
                                acc_ps[:, 0:128 + H], identb[:, :],
                                mext[:, k, 0:128 + H],
                                start=first_mm,
                                stop=(done_w[w] == ntiles_w[w]))
                            first_mm = False
                        t += g

                    if done_w[w] == ntiles_w[w]:
                        dn = smp.tile([128, 8], F32, tag="dn")
                        nc.vector.tensor_scalar(dn[:, 0:H],
                                                acc_ps[:, 128:128 + H],
                                                1e-9, None, op0=OP.add)
                        nc.vector.reciprocal(dn[:, 4:4 + H], dn[:, 0:H])
                        rec_b = (dn[:, 4:4 + H].unsqueeze(2)
                                 .broadcast_to([128, H, D]))
                        hsb = smp.tile([128, 128], BF16, tag="hsb")
                        nc.vector.tensor_tensor(
                            hsb[:].rearrange("p (h d) -> p h d", h=H),
                            acc_ps[:, 0:128]
                            .rearrange("p (h d) -> p h d", h=H),
                            rec_b, op=OP.mult)
                        hT_ps = psp.tile([128, 128], BF16, tag="psb")
                        nc.tensor.matmul(hT_ps[:], hsb[:], identb[:, :],
                                         is_transpose=True,
                                         start=True, stop=True)
                        hTsb = smp.tile([128, 128], BF16, tag="hTsb")
                        nc.vector.tensor_copy(hTsb[:], hT_ps[:])
                        if layer < 2:
                            # hT_next = Minv^T @ hsb^T (undo basis), then relu
                            hT2_ps = psp.tile([128, 128], F32, tag="ps")
                            nc.tensor.matmul(hT2_ps[:], Mt[layer][:],
                                             hTsb[:], start=True, stop=True)
                            nc.scalar.activation(hnext[:, w, :], hT2_ps[:],
                                                 AF.Relu)
                        else:
                            # out = hsb @ Minv (node-major)
                            out_ps = psp.tile([128, 128], F32, tag="ps")
                            nc.tensor.matmul(out_ps[:], hTsb[:],
                                             Mt[layer][:], start=True,
                                             stop=True)
                            osb = smp.tile([128, 128], F32, tag="osb")
                            nc.vector.tensor_copy(osb[:], out_ps[:])
                            nc.sync.dma_start(
                                out_p[:, :].rearrange("(w p) f -> w p f",
                                                      p=128)[w, :, :],
                                osb[:])
    nc.finalize()
    return nc


# ---------------------------------------------------------------------------
# Entry point
# ---------------------------------------------------------------------------

def kernel(features, src, dst, W0, al0, ar0, W1, al1, ar1, W2, al2, ar2):
    out, _ = run_gat(features, src, dst, W0, al0, ar0, W1, al1, ar1,
                     W2, al2, ar2, trace=False)
    return out


def run_gat(features, src, dst, W0, al0, ar0, W1, al1, ar1, W2, al2, ar2,
            trace=False):
    features = np.asarray(features, dtype=np.float32)
    n_nodes = features.shape[0]
    meta = preprocess(src, dst, n_nodes)
    NP, W, npc = meta["NP"], meta["W"], meta["npc"]

    Wm0, A0, Mi0, C0 = pack_weights(np.asarray(W0), al0, ar0)
    Wm1, A1, Mi1, C1 = pack_weights(np.asarray(W1), al1, ar1)
    Wm2, A2, Mi2, C2 = pack_weights(np.asarray(W2), al2, ar2)

    ident = np.eye(128, dtype=np.float32)
    identb = np.eye(128, dtype=np.float32).astype(BF)
    ones1 = np.ones((1, 128), dtype=np.float32)
    onescol = np.ones((128, 1), dtype=np.float32)

    in_maps = []
    for c in range(N_CORES):
        h_c = np.zeros((NP, 128), dtype=np.float32)
        h_c[:npc] = features[c * npc:(c + 1) * npc][meta["perm"][c]]
        in_maps.append({
            "hT0": np.ascontiguousarray(h_c.T).astype(BF),
            "idx": meta["idx_img"][c],
            "valid": meta["valid"][c],
            "W0": Wm0.astype(BF), "W1": Wm1.astype(BF), "W2": Wm2.astype(BF),
            "A0": A0.astype(BF), "A1": A1.astype(BF), "A2": A2.astype(BF),
            "Minv0": Mi0.astype(BF), "Minv1": Mi1.astype(BF),
            "Minv2": Mi2.astype(BF),
            "crep0": C0, "crep1": C1, "crep2": C2,
            "ident": ident, "identb": identb, "ones1": ones1,
            "onescol": onescol,
        })

    nc = build_nc(meta)
    br = run_bass_kernel_spmd(nc, in_maps, list(range(N_CORES)), trace=trace)
    res = br.results

    out = np.empty((n_nodes, 128), dtype=np.float32)
    for c in range(N_CORES):
        o = np.asarray(res[c]["out"])
        out[c * npc:(c + 1) * npc] = o[np.argsort(meta["perm"][c])]
    return out, br


# revision 30
# speedup vs baseline: 2.2774x; 1.3519x over previous
"""GAT (3-layer, DGL-style) on 8 Trainium2 NeuronCores.

Sharding: nodes across the 8 cores (6250 each, padded to 6272 = 49*128),
per-core nodes permuted by descending in-degree.  A "window" is 128 nodes;
a node is pinned to one SBUF partition lane of its window.  Per layer:

  Phase A (node side): featT = W^T @ h^T per window on PE (bf16), er via a
  small second matmul, write 256-byte bf16 feature rows to local DRAM,
  AllGather the 12.8 MB table across cores.

  Phase B (edge side): per window, edge tiles of 128 edges = one in-edge per
  destination partition.  dma_gather fetches 256B bf16 rows (int16 indices;
  the 50176-row table is indexed as two 25088-row halves).  el[src] is
  recomputed per edge on the vector engine (dot with the al vector
  replicated across partitions); er[dst] is a per-partition constant.
  exp(lrelu(s)-C) = max(exp(s-C), exp(0.2*s-C)) on ACT.  Messages (+
  per-head exp columns) are segment-summed by an identity-lhsT bf16 PE
  matmul accumulating into one PSUM bank per window.

C is a per-core bound lrelu(max el + max er) + 3 computed on device; shifting
exp by C instead of the per-segment max changes the reference's +1e-9 epsilon
term by < 1e-3 relative.
"""

import sys

sys.path.insert(0, "/opt/trn_rl_repo")

import ml_dtypes
import numpy as np

import concourse.bass as bass
import concourse.bacc as bacc
import concourse.mybir as mybir
import concourse.tile as tile
from concourse.bass_utils import run_bass_kernel_spmd

F32 = mybir.dt.float32
BF16 = mybir.dt.bfloat16
I16 = mybir.dt.int16
AF = mybir.ActivationFunctionType
OP = mybir.AluOpType
AX = mybir.AxisListType

N_CORES = 8
DIM = 128
ROW = 128              # bf16 elems per table row (256 B, y = feat @ M basis)
CAP = 16               # max tiles per dma_gather call
NEG_SLOPE = 0.2
C_MARGIN = 3.0
HEADS = (4, 4, 1)
BF = ml_dtypes.bfloat16


# ---------------------------------------------------------------------------
# Host-side preprocessing
# ---------------------------------------------------------------------------

def _greedy_halves(src, dst, n_nodes):
    """Assign nodes to table halves (cores 0-3 vs 4-7) so each dst's
    in-neighbors split evenly -> fewer per-window lo/hi tiles."""
    order = np.argsort(src, kind="stable")
    sdst = dst[order]
    starts = np.searchsorted(src[order], np.arange(n_nodes + 1))
    lo = np.zeros(n_nodes, np.int32)
    hi = np.zeros(n_nodes, np.int32)
    assign = np.empty(n_nodes, np.int8)
    cap = n_nodes // 2
    nl = nh = 0
    rng = np.random.default_rng(1)
    for n in rng.permutation(n_nodes):
        d = sdst[starts[n]:starts[n + 1]]
        cl = np.count_nonzero(lo[d] >= hi[d])
        ch = np.count_nonzero(hi[d] >= lo[d])
        if nl >= cap:
            side = 1
        elif nh >= cap:
            side = 0
        elif cl != ch:
            side = 0 if cl < ch else 1
        else:
            side = 0 if nl <= nh else 1
        assign[n] = side
        if side == 0:
            lo[d] += 1
            nl += 1
        else:
            hi[d] += 1
            nh += 1
    return assign


def preprocess(src, dst, n_nodes):
    src = np.asarray(src).astype(np.int64)
    dst = np.asarray(dst).astype(np.int64)
    npc = n_nodes // N_CORES
    NP = ((npc + 127) // 128) * 128
    W = NP // 128
    HALF = 4 * NP
    assert HALF <= 32768, HALF

    assign = _greedy_halves(src, dst, n_nodes)
    degd = np.bincount(dst, minlength=n_nodes)
    core_of = np.empty(n_nodes, dtype=np.int64)
    pos_of = np.empty(n_nodes, dtype=np.int64)
    members = []
    for gstart, base in ((0, 0), (1, 4)):
        mem = np.flatnonzero(assign == gstart)
        mem = mem[np.argsort(-degd[mem], kind="stable")]
        core_of[mem] = base + (np.arange(len(mem)) % 4)
        pos_of[mem] = np.arange(len(mem)) // 4
    for c in range(N_CORES):
        mc = np.flatnonzero(core_of == c)
        members.append(mc[np.argsort(pos_of[mc], kind="stable")])
    core = core_of[dst]
    row_of = core_of * NP + pos_of

    seg_pos = pos_of[dst]
    wv = seg_pos // 128
    pv = seg_pos % 128
    half = (row_of[src] >= HALF).astype(np.int64)

    # occurrence rank within (core, seg, half)
    key = (core * NP + seg_pos) * 2 + half
    order = np.argsort(key, kind="stable")
    ks = key[order]
    starts = np.r_[0, np.flatnonzero(np.diff(ks)) + 1]
    gid = np.zeros(len(ks), dtype=np.int64)
    gid[starts[1:]] = 1
    gid = np.cumsum(gid)
    t_in = np.arange(len(ks)) - starts[gid]
    tv = np.empty(len(ks), dtype=np.int64)
    tv[order] = t_in

    cnt = np.bincount(key, minlength=N_CORES * NP * 2).reshape(
        N_CORES, W, 128, 2)
    T_lo = cnt[:, :, :, 0].max(axis=(0, 2)).astype(np.int64)
    T_hi = cnt[:, :, :, 1].max(axis=(0, 2)).astype(np.int64)

    calls = []
    for w in range(W):
        for hf, T in ((0, int(T_lo[w])), (1, int(T_hi[w]))):
            t = 0
            while t < T:
                nt = min(CAP, T - t)
                calls.append((w, hf, nt))
                t += nt
    gtot = int(T_lo.sum() + T_hi.sum())
    icols = 8 * sum(nt for (_, _, nt) in calls)

    tile_off = np.zeros((W, 2), dtype=np.int64)
    acc = 0
    for w in range(W):
        tile_off[w, 0] = acc
        acc += T_lo[w]
        tile_off[w, 1] = acc
        acc += T_hi[w]

    idx_imgs, valids = [], []
    for c in range(N_CORES):
        m = core == c
        slots_idx = np.zeros((128, gtot), dtype=np.int64)
        slots_val = np.zeros((128, gtot), dtype=np.float32)
        g = tile_off[wv[m], half[m]] + tv[m]
        slots_idx[pv[m], g] = row_of[src[m]] - half[m] * HALF
        slots_val[pv[m], g] = 1.0
        img = np.zeros((16, icols), dtype=np.int16)
        colp = 0
        tile_ptr = {}
        for (w, hf, nt) in calls:
            t0 = tile_ptr.get((w, hf), 0)
            g0 = tile_off[w, hf] + t0
            part = slots_idx[:, g0:g0 + nt]          # [128, nt]
            flat = part.T.reshape(-1)                # j = t*128 + p
            img[:, colp:colp + nt * 8] = flat.reshape(nt * 8, 16).T
            colp += nt * 8
            tile_ptr[(w, hf)] = t0 + nt
        idx_imgs.append(np.ascontiguousarray(np.tile(img, (8, 1))))
        valids.append(slots_val)

    return dict(members=members, calls=calls, T_lo=T_lo, T_hi=T_hi,
                idx_img=idx_imgs, valid=valids, NP=NP, W=W, gtot=gtot,
                icols=icols, npc=npc, HALF=HALF,
                tile_off=tile_off)


def pack_weights(Wl, al, ar):
    """Head-block basis change: y = feat @ M where M's block h has s_h*al_h
    as column 0 (so el_h = y[32h] * al_h[j_h]) and unit columns elsewhere.
    Returns W@M, Minv@A (for er + the C bound), Minv (posthoc undo), and
    crep (the per-head el rescale, replicated to 128 partitions)."""
    H, Dh = Wl.shape[1], Wl.shape[2]
    Wm = np.asarray(Wl, dtype=np.float64).reshape(Wl.shape[0], H * Dh)
    al = np.asarray(al, dtype=np.float64)
    ar = np.asarray(ar, dtype=np.float64)
    M = np.zeros((H * Dh, H * Dh))
    c = np.zeros(H)
    for h in range(H):
        blk = slice(h * Dh, (h + 1) * Dh)
        jh = int(np.argmax(np.abs(al[h])))
        c[h] = al[h][jh]
        Mh = np.zeros((Dh, Dh))
        Mh[:, 0] = al[h] / c[h]
        k = 1
        for j in range(Dh):
            if j != jh:
                Mh[j, k] = 1.0
                k += 1
        M[blk, blk] = Mh
    Minv = np.linalg.inv(M)
    WmM = np.ascontiguousarray((Wm @ M).astype(np.float32))
    A = np.zeros((H * Dh, 8))
    for h in range(H):
        A[h * Dh:(h + 1) * Dh, h] = al[h]
        A[h * Dh:(h + 1) * Dh, 4 + h] = ar[h]
    Ap = np.ascontiguousarray((Minv @ A).astype(np.float32))
    c4 = np.zeros(4, dtype=np.float32)
    c4[:H] = c
    crep = np.ascontiguousarray(np.tile(c4[None, :], (128, 1)))
    Minv128 = np.zeros((128, 128), dtype=np.float32)
    Minv128[:H * Dh, :H * Dh] = Minv
    return WmM, Ap, np.ascontiguousarray(Minv128), crep


# ---------------------------------------------------------------------------
# Device kernel
# ---------------------------------------------------------------------------

def build_nc(meta):
    NP, W, gtot, icols = meta["NP"], meta["W"], meta["gtot"], meta["icols"]
    calls, HALF = meta["calls"], meta["HALF"]
    NTOT = N_CORES * NP
    tile_off = meta["tile_off"]

    nc = bacc.Bacc(None, target_bir_lowering=False, debug=False,
                   num_devices=N_CORES, num_swdge_queues=4)

    hT0 = nc.declare_dram_parameter("hT0", [128, NP], BF16, isOutput=False)
    idx_p = nc.declare_dram_parameter("idx", [128, icols], I16, isOutput=False)
    val_p = nc.declare_dram_parameter("valid", [128, gtot], F32,
                                      isOutput=False)
    Wp = [nc.declare_dram_parameter(f"W{l}", [128, 128], BF16, isOutput=False)
          for l in range(3)]
    Ap = [nc.declare_dram_parameter(f"A{l}", [128, 8], BF16, isOutput=False)
          for l in range(3)]
    Mp = [nc.declare_dram_parameter(f"Minv{l}", [128, 128], BF16,
                                    isOutput=False) for l in range(3)]
    Cp = [nc.declare_dram_parameter(f"crep{l}", [128, 4], F32,
                                    isOutput=False) for l in range(3)]
    ident_p = nc.declare_dram_parameter("ident", [128, 128], F32,
                                        isOutput=False)
    identb_p = nc.declare_dram_parameter("identb", [128, 128], BF16,
                                         isOutput=False)
    ones_p = nc.declare_dram_parameter("ones1", [1, 128], F32, isOutput=False)
    onescol_p = nc.declare_dram_parameter("onescol", [128, 1], F32,
                                          isOutput=False)
    out_p = nc.declare_dram_parameter("out", [NP, 128], F32, isOutput=True)

    with tile.TileContext(nc) as tc:
        with (
            tc.tile_pool(name="const", bufs=1) as constp,
            tc.tile_pool(name="persist", bufs=1) as pers,
            tc.tile_pool(name="featg", bufs=12) as fgp,
            tc.tile_pool(name="mext", bufs=8) as mxp,
            tc.tile_pool(name="small", bufs=6) as smp,
            tc.tile_pool(name="psum", bufs=3, space="PSUM") as psp,
            tc.tile_pool(name="psacc", bufs=2, space="PSUM") as psaccp,
            tc.tile_pool(name="dram", bufs=1, space="DRAM") as dramp,
        ):
            ident = constp.tile([128, 128], F32, tag="ident")
            nc.sync.dma_start(ident[:], ident_p[:, :])
            identb = constp.tile([128, 128], BF16, tag="identb")
            nc.sync.dma_start(identb[:], identb_p[:, :])
            ones1 = constp.tile([1, 128], F32, tag="ones1")
            nc.sync.dma_start(ones1[:], ones_p[:, :])
            onescol = constp.tile([128, 1], F32, tag="onescol")
            nc.sync.dma_start(onescol[:], onescol_p[:, :])
            Wt = [constp.tile([128, 128], BF16, tag=f"W{l}", name=f"Wt{l}")
                  for l in range(3)]
            At = [constp.tile([128, 8], BF16, tag=f"A{l}", name=f"At{l}")
                  for l in range(3)]
            Mt = [constp.tile([128, 128], BF16, tag=f"Minv{l}",
                              name=f"Mt{l}") for l in range(3)]
            Ct = [constp.tile([128, 4], F32, tag=f"crep{l}", name=f"Ct{l}")
                  for l in range(3)]
            for l in range(3):
                nc.sync.dma_start(Wt[l][:], Wp[l][:, :])
                nc.sync.dma_start(At[l][:], Ap[l][:, :])
                nc.sync.dma_start(Mt[l][:], Mp[l][:, :])
                nc.sync.dma_start(Ct[l][:], Cp[l][:, :])
            idx_sb = pers.tile([128, icols], I16, tag="idx")
            nc.sync.dma_start(idx_sb[:], idx_p[:, :])
            valid_sb = pers.tile([128, gtot], F32, tag="valid")
            nc.sync.dma_start(valid_sb[:], val_p[:, :])

            hT = [pers.tile([128, W, 128], BF16, tag=f"hT{i}", name=f"hT{i}")
                  for i in range(2)]
            nc.sync.dma_start(hT[0][:, :, :],
                              hT0[:, :].rearrange("p (w n) -> p w n", w=W))

            elerB = pers.tile([128, W, 8], F32, tag="elerB")
            rowimg = pers.tile([128, W, ROW], BF16, tag="rowimg")

            loc_tbl = dramp.tile([NP, ROW], BF16, tag="loctbl")
            full_tbl = dramp.tile([NTOT, ROW], BF16, tag="fulltbl")

            for layer in range(3):
                H = HEADS[layer]
                D = 128 // H
                hcur, hnext = hT[layer % 2], hT[(layer + 1) % 2]

                # ======== Phase A ========
                for w in range(W):
                    featT_ps = psp.tile([128, 128], F32, tag="ps")
                    nc.tensor.matmul(featT_ps[:], Wt[layer][:],
                                     hcur[:, w, :], start=True, stop=True)
                    featT_sb = smp.tile([128, 128], BF16, tag="featT_sb")
                    nc.vector.tensor_copy(featT_sb[:], featT_ps[:])
                    elerT_ps = psp.tile([8, 128], F32, tag="ps")
                    nc.tensor.matmul(elerT_ps[:], At[layer][:], featT_sb[:],
                                     start=True, stop=True)
                    elerT_sb = smp.tile([8, 128], F32, tag="elerT_sb")
                    nc.vector.tensor_copy(elerT_sb[:], elerT_ps[:])
                    eler_ps = psp.tile([128, 8], F32, tag="ps")
                    nc.tensor.matmul(eler_ps[:], elerT_sb[:],
                                     ident[0:8, 0:8], is_transpose=True,
                                     start=True, stop=True)
                    nc.vector.tensor_copy(elerB[:, w, :], eler_ps[:])
                    feat_ps = psp.tile([128, 128], BF16, tag="psb")
                    nc.tensor.matmul(feat_ps[:], featT_sb[:], identb[:, :],
                                     is_transpose=True, start=True, stop=True)
                    nc.vector.tensor_copy(rowimg[:, w, :], feat_ps[:])
                    nc.sync.dma_start(
                        loc_tbl[:].rearrange("(w p) f -> w p f", p=128)
                        [w, :, :],
                        rowimg[:, w, :])

                # ---- AllGather ----
                nc.gpsimd.collective_compute(
                    "AllGather", OP.bypass,
                    replica_groups=[list(range(N_CORES))],
                    ins=[loc_tbl[:].opt()], outs=[full_tbl[:].opt()])

                # ---- -C = -(lrelu(max el + max er) + margin) ----
                mx = smp.tile([128, 2], F32, tag="mx")
                nc.vector.tensor_reduce(mx[:, 0:1], elerB[:, :, 0:H],
                                        axis=AX.XY, op=OP.max)
                nc.vector.tensor_reduce(mx[:, 1:2], elerB[:, :, 4:4 + H],
                                        axis=AX.XY, op=OP.max)
                mxT_ps = psp.tile([2, 128], F32, tag="ps")
                nc.tensor.matmul(mxT_ps[:], mx[:], ident[:, :],
                                 is_transpose=True, start=True, stop=True)
                mm = smp.tile([2, 1], F32, tag="mm")
                nc.vector.tensor_reduce(mm[:], mxT_ps[:, :], axis=AX.X,
                                        op=OP.max)
                s_ps = psp.tile([1, 1], F32, tag="ps")
                nc.tensor.matmul(s_ps[:], mm[:], onescol[0:2, 0:1],
                                 start=True, stop=True)
                cs = smp.tile([1, 4], F32, tag="cs")
                nc.vector.tensor_copy(cs[:, 0:1], s_ps[:])
                nc.vector.tensor_scalar(cs[:, 1:2], cs[:, 0:1], NEG_SLOPE,
                                        None, op0=OP.mult)
                nc.vector.tensor_tensor(cs[:, 2:3], cs[:, 0:1],
                                        cs[:, 1:2], op=OP.max)
                nc.vector.tensor_scalar(cs[:, 3:4], cs[:, 2:3], -1.0,
                                        -C_MARGIN, op0=OP.mult,
                                        op1=OP.add)
                negC_ps = psp.tile([128, 1], F32, tag="ps")
                nc.tensor.matmul(negC_ps[:], ones1[:], cs[:, 3:4],
                                 start=True, stop=True)
                negC = smp.tile([128, 1], F32, tag="negC")
                nc.vector.tensor_copy(negC[:], negC_ps[:])

                # ======== Phase B ========
                tbl_lo = full_tbl[0:HALF, :]
                tbl_hi = full_tbl[HALF:NTOT, :]
                colp = 0
                tile_ptr = {}
                cur_w = -1
                acc_ps = None
                first_mm = True
                ntiles_w = {w: int(meta["T_lo"][w] + meta["T_hi"][w])
                            for w in range(W)}
                done_w = {w: 0 for w in range(W)}
                qn = 0
                for (w, hf, nt) in calls:
                    if w != cur_w:
                        cur_w = w
                        acc_ps = psaccp.tile([128, 132], F32, tag="acc")
                        first_mm = True
                    t0 = tile_ptr.get((w, hf), 0)
                    tile_ptr[(w, hf)] = t0 + nt
                    g0 = int(tile_off[w, hf]) + t0

                    fg = fgp.tile([128, CAP, ROW], BF16, tag="fg")
                    src_ap = tbl_lo if hf == 0 else tbl_hi
                    nc.gpsimd.dma_gather(
                        fg[:, 0:nt, :], src_ap,
                        idx_sb[:, colp:colp + nt * 8],
                        nt * 128, nt * 128, ROW, elem_step=ROW,
                        single_packet=False, queue_num=qn)
                    qn = (qn + 1) % 4
                    colp += nt * 8

                    t = 0
                    while t < nt:
                        g = min(4, nt - t)
                        sx = smp.tile([128, 4, 4], F32, tag="sx")
                        ux = smp.tile([128, 4, 4], F32, tag="ux")
                        ex = smp.tile([128, 4, 4], F32, tag="exx")
                        elg = smp.tile([128, 4, 4], F32, tag="elg")
                        c_b = (Ct[layer][:, 0:H].unsqueeze(1)
                               .broadcast_to([128, g, H]).unsqueeze(3))
                        nc.vector.tensor_tensor(
                            elg[:, 0:g, 0:H].unsqueeze(3),
                            fg[:, t:t + g, :].rearrange(
                                "p g (h d) -> p g h d", h=H)[:, :, :, 0:1],
                            c_b, op=OP.mult)
                        er_b = (elerB[:, w, 4:4 + H].unsqueeze(1)
                                .broadcast_to([128, g, H]))
                        nc.vector.tensor_tensor(
                            sx[:, 0:g, 0:H], elg[:, 0:g, 0:H],
                            er_b, op=OP.add)
                        nc.scalar.activation(ux[:, 0:g, 0:H], sx[:, 0:g, 0:H],
                                             AF.Exp, bias=negC[:, 0:1],
                                             scale=1.0)
                        nc.scalar.activation(ex[:, 0:g, 0:H], sx[:, 0:g, 0:H],
                                             AF.Exp, bias=negC[:, 0:1],
                                             scale=NEG_SLOPE)
                        val_b = (valid_sb[:, g0 + t:g0 + t + g].unsqueeze(2)
                                 .broadcast_to([128, g, H]))
                        nc.vector.scalar_tensor_tensor(
                            ex[:, 0:g, 0:H], ux[:, 0:g, 0:H], 1.0,
                            ex[:, 0:g, 0:H], op0=OP.mult, op1=OP.max)
                        mext = mxp.tile([128, 4, 132], BF16, tag="mext")
                        nc.vector.tensor_tensor(mext[:, 0:g, 128:128 + H],
                                                ex[:, 0:g, 0:H], val_b,
                                                op=OP.mult)
                        ex_b = (mext[:, 0:g, 128:128 + H].unsqueeze(3)
                                .broadcast_to([128, g, H, D]))
                        nc.vector.tensor_tensor(
                            mext[:, 0:g, 0:128]
                            .rearrange("p g (h d) -> p g h d", h=H),
                            fg[:, t:t + g, 0:128]
                            .rearrange("p g (h d) -> p g h d", h=H),
                            ex_b, op=OP.mult)
                        for k in range(g):
                            done_w[w] += 1
                            nc.tensor.matmul(
                                acc_ps[:, 0:128 + H], identb[:, :],
                                mext[:, k, 0:128 + H],
                                start=first_mm,
                                stop=(done_w[w] == ntiles_w[w]))
                            first_mm = False
                        t += g

                    if done_w[w] == ntiles_w[w]:
                        dn = smp.tile([128, 8], F32, tag="dn")
                        nc.vector.tensor_scalar(dn[:, 0:H],
                                                acc_ps[:, 128:128 + H],
                                                1e-9, None, op0=OP.add)
                        nc.vector.reciprocal(dn[:, 4:4 + H], dn[:, 0:H])
                        rec_b = (dn[:, 4:4 + H].unsqueeze(2)
                                 .broadcast_to([128, H, D]))
                        hsb = smp.tile([128, 128], BF16, tag="hsb")
                        nc.vector.tensor_tensor(
                            hsb[:].rearrange("p (h d) -> p h d", h=H),
                            acc_ps[:, 0:128]
                            .rearrange("p (h d) -> p h d", h=H),
                            rec_b, op=OP.mult)
                        hT_ps = psp.tile([128, 128], BF16, tag="psb")
                        nc.tensor.matmul(hT_ps[:], hsb[:], identb[:, :],
                                         is_transpose=True,
                                         start=True, stop=True)
                        hTsb = smp.tile([128, 128], BF16, tag="hTsb")
                        nc.vector.tensor_copy(hTsb[:], hT_ps[:])
                        if layer < 2:
                            # hT_next = Minv^T @ hsb^T (undo basis), then relu
                            hT2_ps = psp.tile([128, 128], F32, tag="ps")
                            nc.tensor.matmul(hT2_ps[:], Mt[layer][:],
                                             hTsb[:], start=True, stop=True)
                            nc.scalar.activation(hnext[:, w, :], hT2_ps[:],
                                                 AF.Relu)
                        else:
                            # out = hsb @ Minv (node-major)
                            out_ps = psp.tile([128, 128], F32, tag="ps")
                            nc.tensor.matmul(out_ps[:], hTsb[:],
                                             Mt[layer][:], start=True,
                                             stop=True)
                            osb = smp.tile([128, 128], F32, tag="osb")
                            nc.vector.tensor_copy(osb[:], out_ps[:])
                            nc.sync.dma_start(
                                out_p[:, :].rearrange("(w p) f -> w p f",
                                                      p=128)[w, :, :],
                                osb[:])
    nc.finalize()
    return nc


# ---------------------------------------------------------------------------
# Entry point
# ---------------------------------------------------------------------------

def kernel(features, src, dst, W0, al0, ar0, W1, al1, ar1, W2, al2, ar2):
    out, _ = run_gat(features, src, dst, W0, al0, ar0, W1, al1, ar1,
                     W2, al2, ar2, trace=False)
    return out


def run_gat(features, src, dst, W0, al0, ar0, W1, al1, ar1, W2, al2, ar2,
            trace=False):
    features = np.asarray(features, dtype=np.float32)
    n_nodes = features.shape[0]
    meta = preprocess(src, dst, n_nodes)
    NP, W, npc = meta["NP"], meta["W"], meta["npc"]

    Wm0, A0, Mi0, C0 = pack_weights(np.asarray(W0), al0, ar0)
    Wm1, A1, Mi1, C1 = pack_weights(np.asarray(W1), al1, ar1)
    Wm2, A2, Mi2, C2 = pack_weights(np.asarray(W2), al2, ar2)

    ident = np.eye(128, dtype=np.float32)
    identb = np.eye(128, dtype=np.float32).astype(BF)
    ones1 = np.ones((1, 128), dtype=np.float32)
    onescol = np.ones((128, 1), dtype=np.float32)

    in_maps = []
    for c in range(N_CORES):
        mem = meta["members"][c]
        h_c = np.zeros((NP, 128), dtype=np.float32)
        h_c[:len(mem)] = features[mem]
        in_maps.append({
            "hT0": np.ascontiguousarray(h_c.T).astype(BF),
            "idx": meta["idx_img"][c],
            "valid": meta["valid"][c],
            "W0": Wm0.astype(BF), "W1": Wm1.astype(BF), "W2": Wm2.astype(BF),
            "A0": A0.astype(BF), "A1": A1.astype(BF), "A2": A2.astype(BF),
            "Minv0": Mi0.astype(BF), "Minv1": Mi1.astype(BF),
            "Minv2": Mi2.astype(BF),
            "crep0": C0, "crep1": C1, "crep2": C2,
            "ident": ident, "identb": identb, "ones1": ones1,
            "onescol": onescol,
        })

    nc = build_nc(meta)
    br = run_bass_kernel_spmd(nc, in_maps, list(range(N_CORES)), trace=trace)
    res = br.results

    out = np.empty((n_nodes, 128), dtype=np.float32)
    for c in range(N_CORES):
        mem = meta["members"][c]
        o = np.asarray(res[c]["out"])
        out[mem] = o[:len(mem)]
    return out, br


# revision 43
# speedup vs baseline: 3.2946x; 1.4466x over previous
"""GAT (3-layer, DGL-style) on 8 Trainium2 NeuronCores.

Sharding: nodes across the 8 cores (6250 each, padded to 6272 = 49*128),
per-core nodes permuted by descending in-degree.  A "window" is 128 nodes;
a node is pinned to one SBUF partition lane of its window.  Per layer:

  Phase A (node side): featT = W^T @ h^T per window on PE (bf16), er via a
  small second matmul, write 256-byte bf16 feature rows to local DRAM,
  AllGather the 12.8 MB table across cores.

  Phase B (edge side): per window, edge tiles of 128 edges = one in-edge per
  destination partition.  dma_gather fetches 256B bf16 rows (int16 indices;
  the 50176-row table is indexed as two 25088-row halves).  el[src] is
  recomputed per edge on the vector engine (dot with the al vector
  replicated across partitions); er[dst] is a per-partition constant.
  exp(lrelu(s)-C) = max(exp(s-C), exp(0.2*s-C)) on ACT.  Messages (+
  per-head exp columns) are segment-summed by an identity-lhsT bf16 PE
  matmul accumulating into one PSUM bank per window.

C is a per-core bound lrelu(max el + max er) + 3 computed on device; shifting
exp by C instead of the per-segment max changes the reference's +1e-9 epsilon
term by < 1e-3 relative.
"""

import sys

sys.path.insert(0, "/opt/trn_rl_repo")

import ml_dtypes
import numpy as np

import concourse.bass as bass
import concourse.bacc as bacc
import concourse.mybir as mybir
import concourse.tile as tile
from concourse.bass_utils import run_bass_kernel_spmd

F32 = mybir.dt.float32
BF16 = mybir.dt.bfloat16
I16 = mybir.dt.int16
AF = mybir.ActivationFunctionType
OP = mybir.AluOpType
AX = mybir.AxisListType

N_CORES = 8
DIM = 128
ROW = 128              # bf16 elems per table row (256 B, y = feat @ M basis)
CAP = 16               # max tiles per dma_gather call
NEG_SLOPE = 0.2
C_MARGIN = 3.0
HEADS = (4, 4, 1)
BF = ml_dtypes.bfloat16


# ---------------------------------------------------------------------------
# Host-side preprocessing
# ---------------------------------------------------------------------------

def _greedy_halves(src, dst, n_nodes):
    """Assign nodes to table halves (cores 0-3 vs 4-7) so each dst's
    in-neighbors split evenly -> fewer per-window lo/hi tiles."""
    order = np.argsort(src, kind="stable")
    sdst = dst[order]
    starts = np.searchsorted(src[order], np.arange(n_nodes + 1))
    lo = np.zeros(n_nodes, np.int32)
    hi = np.zeros(n_nodes, np.int32)
    assign = np.empty(n_nodes, np.int8)
    cap = n_nodes // 2
    nl = nh = 0
    rng = np.random.default_rng(1)
    for n in rng.permutation(n_nodes):
        d = sdst[starts[n]:starts[n + 1]]
        cl = np.count_nonzero(lo[d] >= hi[d])
        ch = np.count_nonzero(hi[d] >= lo[d])
        if nl >= cap:
            side = 1
        elif nh >= cap:
            side = 0
        elif cl != ch:
            side = 0 if cl < ch else 1
        else:
            side = 0 if nl <= nh else 1
        assign[n] = side
        if side == 0:
            lo[d] += 1
            nl += 1
        else:
            hi[d] += 1
            nh += 1
    return assign


def preprocess(src, dst, n_nodes):
    src = np.asarray(src).astype(np.int64)
    dst = np.asarray(dst).astype(np.int64)
    npc = n_nodes // N_CORES
    NP = ((npc + 127) // 128) * 128
    W = NP // 128
    HALF = 4 * NP
    assert HALF <= 32768, HALF

    assign = _greedy_halves(src, dst, n_nodes)
    degd = np.bincount(dst, minlength=n_nodes)
    core_of = np.empty(n_nodes, dtype=np.int64)
    pos_of = np.empty(n_nodes, dtype=np.int64)
    members = []
    for gstart, base in ((0, 0), (1, 4)):
        mem = np.flatnonzero(assign == gstart)
        mem = mem[np.argsort(-degd[mem], kind="stable")]
        core_of[mem] = base + (np.arange(len(mem)) % 4)
        pos_of[mem] = np.arange(len(mem)) // 4
    for c in range(N_CORES):
        mc = np.flatnonzero(core_of == c)
        members.append(mc[np.argsort(pos_of[mc], kind="stable")])
    core = core_of[dst]
    row_of = core_of * NP + pos_of

    seg_pos = pos_of[dst]
    wv = seg_pos // 128
    pv = seg_pos % 128
    half = (row_of[src] >= HALF).astype(np.int64)

    # dense slot packing: occurrence rank within (core, window, half);
    # any slot may serve any dst of the window (PT matrix in the kernel)
    key = (core * W + wv) * 2 + half
    order = np.argsort(key, kind="stable")
    ks = key[order]
    starts = np.r_[0, np.flatnonzero(np.diff(ks)) + 1]
    gid = np.zeros(len(ks), dtype=np.int64)
    gid[starts[1:]] = 1
    gid = np.cumsum(gid)
    t_in = np.arange(len(ks)) - starts[gid]
    sv = np.empty(len(ks), dtype=np.int64)
    sv[order] = t_in                       # dense slot within (c, w, hf)

    cnt3 = np.bincount(key, minlength=N_CORES * W * 2).reshape(
        N_CORES, W, 2)
    Tm = ((cnt3 + 127) // 128).max(axis=0).astype(np.int64)
    T_lo, T_hi = Tm[:, 0], Tm[:, 1]

    calls = []
    for w in range(W):
        for hf, T in ((0, int(T_lo[w])), (1, int(T_hi[w]))):
            t = 0
            while t < T:
                nt = min(CAP, T - t)
                calls.append((w, hf, nt))
                t += nt
    gtot = int(T_lo.sum() + T_hi.sum())
    icols = 8 * sum(nt for (_, _, nt) in calls)

    tile_off = np.zeros((W, 2), dtype=np.int64)
    acc = 0
    for w in range(W):
        tile_off[w, 0] = acc
        acc += T_lo[w]
        tile_off[w, 1] = acc
        acc += T_hi[w]

    idx_imgs, dstids = [], []
    for c in range(N_CORES):
        m = core == c
        slots_idx = np.zeros((128, gtot), dtype=np.int64)
        slots_did = np.full((128, gtot), 200.0, dtype=np.float32)
        g = tile_off[wv[m], half[m]] + sv[m] // 128
        lane = sv[m] % 128
        slots_idx[lane, g] = row_of[src[m]] - half[m] * HALF
        slots_did[lane, g] = pv[m].astype(np.float32)
        img = np.zeros((16, icols), dtype=np.int16)
        colp = 0
        tile_ptr = {}
        for (w, hf, nt) in calls:
            t0 = tile_ptr.get((w, hf), 0)
            g0 = tile_off[w, hf] + t0
            part = slots_idx[:, g0:g0 + nt]          # [128, nt]
            flat = part.T.reshape(-1)                # j = t*128 + p
            img[:, colp:colp + nt * 8] = flat.reshape(nt * 8, 16).T
            colp += nt * 8
            tile_ptr[(w, hf)] = t0 + nt
        idx_imgs.append(np.ascontiguousarray(np.tile(img, (8, 1))))
        dstids.append(slots_did)

    return dict(members=members, calls=calls, T_lo=T_lo, T_hi=T_hi,
                idx_img=idx_imgs, dstid=dstids, NP=NP, W=W, gtot=gtot,
                icols=icols, npc=npc, HALF=HALF,
                tile_off=tile_off)


def pack_weights(Wl, al, ar):
    """Head-block basis change: y = feat @ M where M's block h has s_h*al_h
    as column 0 (so el_h = y[32h] * al_h[j_h]) and unit columns elsewhere.
    Returns W@M, Minv@A (for er + the C bound), Minv (posthoc undo), and
    crep (the per-head el rescale, replicated to 128 partitions)."""
    H, Dh = Wl.shape[1], Wl.shape[2]
    Wm = np.asarray(Wl, dtype=np.float64).reshape(Wl.shape[0], H * Dh)
    al = np.asarray(al, dtype=np.float64)
    ar = np.asarray(ar, dtype=np.float64)
    M = np.zeros((H * Dh, H * Dh))
    c = np.zeros(H)
    for h in range(H):
        blk = slice(h * Dh, (h + 1) * Dh)
        jh = int(np.argmax(np.abs(al[h])))
        c[h] = al[h][jh]
        Mh = np.zeros((Dh, Dh))
        Mh[:, 0] = al[h] / c[h]
        k = 1
        for j in range(Dh):
            if j != jh:
                Mh[j, k] = 1.0
                k += 1
        M[blk, blk] = Mh
    Minv = np.linalg.inv(M)
    WmM = np.ascontiguousarray((Wm @ M).astype(np.float32))
    A = np.zeros((H * Dh, 8))
    for h in range(H):
        A[h * Dh:(h + 1) * Dh, h] = al[h]
        A[h * Dh:(h + 1) * Dh, 4 + h] = ar[h]
    Ap = np.ascontiguousarray((Minv @ A).astype(np.float32))
    c4 = np.zeros(4, dtype=np.float32)
    c4[:H] = c
    crep = np.ascontiguousarray(np.tile(c4[None, :], (128, 1)))
    Minv128 = np.zeros((128, 128), dtype=np.float32)
    Minv128[:H * Dh, :H * Dh] = Minv
    return WmM, Ap, np.ascontiguousarray(Minv128), crep


# ---------------------------------------------------------------------------
# Device kernel
# ---------------------------------------------------------------------------

def build_nc(meta):
    NP, W, gtot, icols = meta["NP"], meta["W"], meta["gtot"], meta["icols"]
    calls, HALF = meta["calls"], meta["HALF"]
    NTOT = N_CORES * NP
    tile_off = meta["tile_off"]

    nc = bacc.Bacc(None, target_bir_lowering=False, debug=False,
                   num_devices=N_CORES, num_swdge_queues=4)

    hT0 = nc.declare_dram_parameter("hT0", [128, NP], BF16, isOutput=False)
    idx_p = nc.declare_dram_parameter("idx", [128, icols], I16, isOutput=False)
    did_p = nc.declare_dram_parameter("dstid", [128, gtot], BF16,
                                      isOutput=False)
    iotab_p = nc.declare_dram_parameter("iotab", [128, 128], BF16,
                                        isOutput=False)
    Wp = [nc.declare_dram_parameter(f"W{l}", [128, 128], BF16, isOutput=False)
          for l in range(3)]
    Ap = [nc.declare_dram_parameter(f"A{l}", [128, 8], BF16, isOutput=False)
          for l in range(3)]
    Mp = [nc.declare_dram_parameter(f"Minv{l}", [128, 128], BF16,
                                    isOutput=False) for l in range(3)]
    Cp = [nc.declare_dram_parameter(f"crep{l}", [128, 4], F32,
                                    isOutput=False) for l in range(3)]
    ident_p = nc.declare_dram_parameter("ident", [128, 128], F32,
                                        isOutput=False)
    identb_p = nc.declare_dram_parameter("identb", [128, 128], BF16,
                                         isOutput=False)
    ones_p = nc.declare_dram_parameter("ones1", [1, 128], F32, isOutput=False)
    onescol_p = nc.declare_dram_parameter("onescol", [128, 1], F32,
                                          isOutput=False)
    out_p = nc.declare_dram_parameter("out", [NP, 128], F32, isOutput=True)

    with tile.TileContext(nc) as tc:
        with (
            tc.tile_pool(name="const", bufs=1) as constp,
            tc.tile_pool(name="persist", bufs=1) as pers,
            tc.tile_pool(name="featg", bufs=12) as fgp,
            tc.tile_pool(name="mext", bufs=8) as mxp,
            tc.tile_pool(name="ptpool", bufs=8) as ptp,
            tc.tile_pool(name="small", bufs=6) as smp,
            tc.tile_pool(name="psum", bufs=3, space="PSUM") as psp,
            tc.tile_pool(name="psacc", bufs=2, space="PSUM") as psaccp,
            tc.tile_pool(name="dram", bufs=1, space="DRAM") as dramp,
        ):
            ident = constp.tile([128, 128], F32, tag="ident")
            nc.sync.dma_start(ident[:], ident_p[:, :])
            identb = constp.tile([128, 128], BF16, tag="identb")
            nc.sync.dma_start(identb[:], identb_p[:, :])
            ones1 = constp.tile([1, 128], F32, tag="ones1")
            nc.sync.dma_start(ones1[:], ones_p[:, :])
            onescol = constp.tile([128, 1], F32, tag="onescol")
            nc.sync.dma_start(onescol[:], onescol_p[:, :])
            Wt = [constp.tile([128, 128], BF16, tag=f"W{l}", name=f"Wt{l}")
                  for l in range(3)]
            At = [constp.tile([128, 8], BF16, tag=f"A{l}", name=f"At{l}")
                  for l in range(3)]
            Mt = [constp.tile([128, 128], BF16, tag=f"Minv{l}",
                              name=f"Mt{l}") for l in range(3)]
            Ct = [constp.tile([128, 4], F32, tag=f"crep{l}", name=f"Ct{l}")
                  for l in range(3)]
            for l in range(3):
                nc.sync.dma_start(Wt[l][:], Wp[l][:, :])
                nc.sync.dma_start(At[l][:], Ap[l][:, :])
                nc.sync.dma_start(Mt[l][:], Mp[l][:, :])
                nc.sync.dma_start(Ct[l][:], Cp[l][:, :])
            idx_sb = pers.tile([128, icols], I16, tag="idx")
            nc.sync.dma_start(idx_sb[:], idx_p[:, :])
            dstid_sb = pers.tile([128, gtot], BF16, tag="dstid")
            nc.sync.dma_start(dstid_sb[:], did_p[:, :])
            iotab = constp.tile([128, 128], BF16, tag="iotab")
            nc.sync.dma_start(iotab[:], iotab_p[:, :])

            hT = [pers.tile([128, W, 128], BF16, tag=f"hT{i}", name=f"hT{i}")
                  for i in range(2)]
            nc.sync.dma_start(hT[0][:, :, :],
                              hT0[:, :].rearrange("p (w n) -> p w n", w=W))

            elerB = pers.tile([128, W, 8], F32, tag="elerB")
            erBb = pers.tile([128, W, 4], BF16, tag="erBb")
            rowimg = pers.tile([128, W, ROW], BF16, tag="rowimg")

            loc_tbl = dramp.tile([NP, ROW], BF16, tag="loctbl")
            full_tbl = dramp.tile([NTOT, ROW], BF16, tag="fulltbl")

            for layer in range(3):
                H = HEADS[layer]
                D = 128 // H
                hcur, hnext = hT[layer % 2], hT[(layer + 1) % 2]

                # ======== Phase A ========
                for w in range(W):
                    featT_ps = psp.tile([128, 128], F32, tag="ps")
                    nc.tensor.matmul(featT_ps[:], Wt[layer][:],
                                     hcur[:, w, :], start=True, stop=True)
                    featT_sb = smp.tile([128, 128], BF16, tag="featT_sb")
                    nc.vector.tensor_copy(featT_sb[:], featT_ps[:])
                    elerT_ps = psp.tile([8, 128], F32, tag="ps")
                    nc.tensor.matmul(elerT_ps[:], At[layer][:], featT_sb[:],
                                     start=True, stop=True)
                    elerT_sb = smp.tile([8, 128], F32, tag="elerT_sb")
                    nc.vector.tensor_copy(elerT_sb[:], elerT_ps[:])
                    eler_ps = psp.tile([128, 8], F32, tag="ps")
                    nc.tensor.matmul(eler_ps[:], elerT_sb[:],
                                     ident[0:8, 0:8], is_transpose=True,
                                     start=True, stop=True)
                    nc.vector.tensor_copy(elerB[:, w, :], eler_ps[:])
                    nc.vector.tensor_copy(erBb[:, w, 0:H],
                                          eler_ps[:, 4:4 + H])
                    feat_ps = psp.tile([128, 128], BF16, tag="psb")
                    nc.tensor.matmul(feat_ps[:], featT_sb[:], identb[:, :],
                                     is_transpose=True, start=True, stop=True)
                    nc.vector.tensor_copy(rowimg[:, w, :], feat_ps[:])
                    nc.sync.dma_start(
                        loc_tbl[:].rearrange("(w p) f -> w p f", p=128)
                        [w, :, :],
                        rowimg[:, w, :])

                # ---- AllGather ----
                nc.gpsimd.collective_compute(
                    "AllGather", OP.bypass,
                    replica_groups=[list(range(N_CORES))],
                    ins=[loc_tbl[:].opt()], outs=[full_tbl[:].opt()])

                # ---- -C = -(lrelu(max el + max er) + margin) ----
                mx = smp.tile([128, 2], F32, tag="mx")
                nc.vector.tensor_reduce(mx[:, 0:1], elerB[:, :, 0:H],
                                        axis=AX.XY, op=OP.max)
                nc.vector.tensor_reduce(mx[:, 1:2], elerB[:, :, 4:4 + H],
                                        axis=AX.XY, op=OP.max)
                mxT_ps = psp.tile([2, 128], F32, tag="ps")
                nc.tensor.matmul(mxT_ps[:], mx[:], ident[:, :],
                                 is_transpose=True, start=True, stop=True)
                mm = smp.tile([2, 1], F32, tag="mm")
                nc.vector.tensor_reduce(mm[:], mxT_ps[:, :], axis=AX.X,
                                        op=OP.max)
                s_ps = psp.tile([1, 1], F32, tag="ps")
                nc.tensor.matmul(s_ps[:], mm[:], onescol[0:2, 0:1],
                                 start=True, stop=True)
                cs = smp.tile([1, 4], F32, tag="cs")
                nc.vector.tensor_copy(cs[:, 0:1], s_ps[:])
                nc.vector.tensor_scalar(cs[:, 1:2], cs[:, 0:1], NEG_SLOPE,
                                        None, op0=OP.mult)
                nc.vector.tensor_tensor(cs[:, 2:3], cs[:, 0:1],
                                        cs[:, 1:2], op=OP.max)
                nc.vector.tensor_scalar(cs[:, 3:4], cs[:, 2:3], -1.0,
                                        -C_MARGIN, op0=OP.mult,
                                        op1=OP.add)
                negC_ps = psp.tile([128, 1], F32, tag="ps")
                nc.tensor.matmul(negC_ps[:], ones1[:], cs[:, 3:4],
                                 start=True, stop=True)
                negC = smp.tile([128, 1], F32, tag="negC")
                nc.vector.tensor_copy(negC[:], negC_ps[:])

                # ======== Phase B ========
                tbl_lo = full_tbl[0:HALF, :]
                tbl_hi = full_tbl[HALF:NTOT, :]
                colp = 0
                tile_ptr = {}
                cur_w = -1
                acc_ps = None
                first_mm = True
                ntiles_w = {w: int(meta["T_lo"][w] + meta["T_hi"][w])
                            for w in range(W)}
                done_w = {w: 0 for w in range(W)}
                qn = 0
                for (w, hf, nt) in calls:
                    if w != cur_w:
                        cur_w = w
                        acc_ps = psaccp.tile([128, 132], F32, tag="acc")
                        first_mm = True
                    t0 = tile_ptr.get((w, hf), 0)
                    tile_ptr[(w, hf)] = t0 + nt
                    g0 = int(tile_off[w, hf]) + t0

                    fg = fgp.tile([128, CAP, ROW], BF16, tag="fg")
                    src_ap = tbl_lo if hf == 0 else tbl_hi
                    nc.gpsimd.dma_gather(
                        fg[:, 0:nt, :], src_ap,
                        idx_sb[:, colp:colp + nt * 8],
                        nt * 128, nt * 128, ROW, elem_step=ROW,
                        single_packet=False, queue_num=qn)
                    qn = (qn + 1) % 4
                    colp += nt * 8

                    t = 0
                    while t < nt:
                        g = min(4, nt - t)
                        # PT[slot, dst] = (dstid[slot] == dst), per tile
                        pt4 = ptp.tile([128, 4, 128], BF16, tag="pt4")
                        did_b = (dstid_sb[:, g0 + t:g0 + t + g].unsqueeze(2)
                                 .broadcast_to([128, g, 128]))
                        iot_b = (iotab[:].unsqueeze(1)
                                 .broadcast_to([128, g, 128]))
                        nc.vector.tensor_tensor(pt4[:, 0:g, :], did_b,
                                                iot_b, op=OP.is_equal)
                        # er aligned to slots: er_slot = PT^T-permute of er
                        er4_ps = psp.tile([128, 4, 4], F32, tag="ps")
                        for k in range(g):
                            ptT_ps = psp.tile([128, 128], BF16, tag="psb")
                            nc.tensor.matmul(ptT_ps[:], pt4[:, k, :],
                                             identb[:, :], is_transpose=True,
                                             start=True, stop=True)
                            ptT_sb = smp.tile([128, 128], BF16, tag="ptT")
                            nc.vector.tensor_copy(ptT_sb[:], ptT_ps[:])
                            nc.tensor.matmul(er4_ps[:, k, 0:H], ptT_sb[:],
                                             erBb[:, w, 0:H],
                                             start=True, stop=True)
                        sx = smp.tile([128, 4, 4], F32, tag="sx")
                        ux = smp.tile([128, 4, 4], F32, tag="ux")
                        ex = smp.tile([128, 4, 4], F32, tag="exx")
                        elg = smp.tile([128, 4, 4], F32, tag="elg")
                        c_b = (Ct[layer][:, 0:H].unsqueeze(1)
                               .broadcast_to([128, g, H]).unsqueeze(3))
                        nc.vector.tensor_tensor(
                            elg[:, 0:g, 0:H].unsqueeze(3),
                            fg[:, t:t + g, :].rearrange(
                                "p g (h d) -> p g h d", h=H)[:, :, :, 0:1],
                            c_b, op=OP.mult)
                        nc.vector.tensor_tensor(
                            sx[:, 0:g, 0:H], elg[:, 0:g, 0:H],
                            er4_ps[:, 0:g, 0:H], op=OP.add)
                        nc.scalar.activation(ux[:, 0:g, 0:H], sx[:, 0:g, 0:H],
                                             AF.Exp, bias=negC[:, 0:1],
                                             scale=1.0)
                        nc.scalar.activation(ex[:, 0:g, 0:H], sx[:, 0:g, 0:H],
                                             AF.Exp, bias=negC[:, 0:1],
                                             scale=NEG_SLOPE)
                        mext = mxp.tile([128, 4, 132], BF16, tag="mext")
                        nc.vector.scalar_tensor_tensor(
                            mext[:, 0:g, 128:128 + H], ux[:, 0:g, 0:H], 1.0,
                            ex[:, 0:g, 0:H], op0=OP.mult, op1=OP.max)
                        ex_b = (mext[:, 0:g, 128:128 + H].unsqueeze(3)
                                .broadcast_to([128, g, H, D]))
                        nc.vector.tensor_tensor(
                            mext[:, 0:g, 0:128]
                            .rearrange("p g (h d) -> p g h d", h=H),
                            fg[:, t:t + g, 0:128]
                            .rearrange("p g (h d) -> p g h d", h=H),
                            ex_b, op=OP.mult)
                        for k in range(g):
                            done_w[w] += 1
                            nc.tensor.matmul(
                                acc_ps[:, 0:128 + H], pt4[:, k, :],
                                mext[:, k, 0:128 + H],
                                start=first_mm,
                                stop=(done_w[w] == ntiles_w[w]))
                            first_mm = False
                        t += g

                    if done_w[w] == ntiles_w[w]:
                        dn = smp.tile([128, 8], F32, tag="dn")
                        nc.vector.tensor_scalar(dn[:, 0:H],
                                                acc_ps[:, 128:128 + H],
                                                1e-9, None, op0=OP.add)
                        nc.vector.reciprocal(dn[:, 4:4 + H], dn[:, 0:H])
                        rec_b = (dn[:, 4:4 + H].unsqueeze(2)
                                 .broadcast_to([128, H, D]))
                        hsb = smp.tile([128, 128], BF16, tag="hsb")
                        nc.vector.tensor_tensor(
                            hsb[:].rearrange("p (h d) -> p h d", h=H),
                            acc_ps[:, 0:128]
                            .rearrange("p (h d) -> p h d", h=H),
                            rec_b, op=OP.mult)
                        hT_ps = psp.tile([128, 128], BF16, tag="psb")
                        nc.tensor.matmul(hT_ps[:], hsb[:], identb[:, :],
                                         is_transpose=True,
                                         start=True, stop=True)
                        hTsb = smp.tile([128, 128], BF16, tag="hTsb")
                        nc.vector.tensor_copy(hTsb[:], hT_ps[:])
                        if layer < 2:
                            # hT_next = Minv^T @ hsb^T (undo basis), then relu
                            hT2_ps = psp.tile([128, 128], F32, tag="ps")
                            nc.tensor.matmul(hT2_ps[:], Mt[layer][:],
                                             hTsb[:], start=True, stop=True)
                            nc.scalar.activation(hnext[:, w, :], hT2_ps[:],
                                                 AF.Relu)
                        else:
                            # out = hsb @ Minv (node-major)
                            out_ps = psp.tile([128, 128], F32, tag="ps")
                            nc.tensor.matmul(out_ps[:], hTsb[:],
                                             Mt[layer][:], start=True,
                                             stop=True)
                            osb = smp.tile([128, 128], F32, tag="osb")
                            nc.vector.tensor_copy(osb[:], out_ps[:])
                            nc.sync.dma_start(
                                out_p[:, :].rearrange("(w p) f -> w p f",
                                                      p=128)[w, :, :],
                                osb[:])
    nc.finalize()
    return nc


# ---------------------------------------------------------------------------
# Entry point
# ---------------------------------------------------------------------------

def kernel(features, src, dst, W0, al0, ar0, W1, al1, ar1, W2, al2, ar2):
    out, _ = run_gat(features, src, dst, W0, al0, ar0, W1, al1, ar1,
                     W2, al2, ar2, trace=False)
    return out


def run_gat(features, src, dst, W0, al0, ar0, W1, al1, ar1, W2, al2, ar2,
            trace=False):
    features = np.asarray(features, dtype=np.float32)
    n_nodes = features.shape[0]
    meta = preprocess(src, dst, n_nodes)
    NP, W, npc = meta["NP"], meta["W"], meta["npc"]

    Wm0, A0, Mi0, C0 = pack_weights(np.asarray(W0), al0, ar0)
    Wm1, A1, Mi1, C1 = pack_weights(np.asarray(W1), al1, ar1)
    Wm2, A2, Mi2, C2 = pack_weights(np.asarray(W2), al2, ar2)

    ident = np.eye(128, dtype=np.float32)
    identb = np.eye(128, dtype=np.float32).astype(BF)
    ones1 = np.ones((1, 128), dtype=np.float32)
    onescol = np.ones((128, 1), dtype=np.float32)

    in_maps = []
    for c in range(N_CORES):
        mem = meta["members"][c]
        h_c = np.zeros((NP, 128), dtype=np.float32)
        h_c[:len(mem)] = features[mem]
        in_maps.append({
            "hT0": np.ascontiguousarray(h_c.T).astype(BF),
            "idx": meta["idx_img"][c],
            "dstid": meta["dstid"][c].astype(BF),
            "iotab": np.tile(np.arange(128, dtype=np.float32)[None, :],
                             (128, 1)).astype(BF),
            "W0": Wm0.astype(BF), "W1": Wm1.astype(BF), "W2": Wm2.astype(BF),
            "A0": A0.astype(BF), "A1": A1.astype(BF), "A2": A2.astype(BF),
            "Minv0": Mi0.astype(BF), "Minv1": Mi1.astype(BF),
            "Minv2": Mi2.astype(BF),
            "crep0": C0, "crep1": C1, "crep2": C2,
            "ident": ident, "identb": identb, "ones1": ones1,
            "onescol": onescol,
        })

    nc = build_nc(meta)
    br = run_bass_kernel_spmd(nc, in_maps, list(range(N_CORES)), trace=trace)
    res = br.results

    out = np.empty((n_nodes, 128), dtype=np.float32)
    for c in range(N_CORES):
        mem = meta["members"][c]
        o = np.asarray(res[c]["out"])
        out[mem] = o[:len(mem)]
    return out, br


# revision 45
# speedup vs baseline: 3.3398x; 1.0137x over previous
"""GAT (3-layer, DGL-style) on 8 Trainium2 NeuronCores.

Sharding: nodes across the 8 cores (6250 each, padded to 6272 = 49*128),
per-core nodes permuted by descending in-degree.  A "window" is 128 nodes;
a node is pinned to one SBUF partition lane of its window.  Per layer:

  Phase A (node side): featT = W^T @ h^T per window on PE (bf16), er via a
  small second matmul, write 256-byte bf16 feature rows to local DRAM,
  AllGather the 12.8 MB table across cores.

  Phase B (edge side): per window, edge tiles of 128 edges = one in-edge per
  destination partition.  dma_gather fetches 256B bf16 rows (int16 indices;
  the 50176-row table is indexed as two 25088-row halves).  el[src] is
  recomputed per edge on the vector engine (dot with the al vector
  replicated across partitions); er[dst] is a per-partition constant.
  exp(lrelu(s)-C) = max(exp(s-C), exp(0.2*s-C)) on ACT.  Messages (+
  per-head exp columns) are segment-summed by an identity-lhsT bf16 PE
  matmul accumulating into one PSUM bank per window.

C is a per-core bound lrelu(max el + max er) + 3 computed on device; shifting
exp by C instead of the per-segment max changes the reference's +1e-9 epsilon
term by < 1e-3 relative.
"""

import sys

sys.path.insert(0, "/opt/trn_rl_repo")

import ml_dtypes
import numpy as np

import concourse.bass as bass
import concourse.bacc as bacc
import concourse.mybir as mybir
import concourse.tile as tile
from concourse.bass_utils import run_bass_kernel_spmd

F32 = mybir.dt.float32
BF16 = mybir.dt.bfloat16
I16 = mybir.dt.int16
AF = mybir.ActivationFunctionType
OP = mybir.AluOpType
AX = mybir.AxisListType

N_CORES = 8
DIM = 128
ROW = 128              # bf16 elems per table row (256 B, y = feat @ M basis)
CAP = 16               # max tiles per dma_gather call
NEG_SLOPE = 0.2
C_MARGIN = 3.0
HEADS = (4, 4, 1)
BF = ml_dtypes.bfloat16


# ---------------------------------------------------------------------------
# Host-side preprocessing
# ---------------------------------------------------------------------------

def _greedy_halves(src, dst, n_nodes):
    """Assign nodes to table halves (cores 0-3 vs 4-7) so each dst's
    in-neighbors split evenly -> fewer per-window lo/hi tiles."""
    order = np.argsort(src, kind="stable")
    sdst = dst[order]
    starts = np.searchsorted(src[order], np.arange(n_nodes + 1))
    lo = np.zeros(n_nodes, np.int32)
    hi = np.zeros(n_nodes, np.int32)
    assign = np.empty(n_nodes, np.int8)
    cap = n_nodes // 2
    nl = nh = 0
    rng = np.random.default_rng(1)
    for n in rng.permutation(n_nodes):
        d = sdst[starts[n]:starts[n + 1]]
        cl = np.count_nonzero(lo[d] >= hi[d])
        ch = np.count_nonzero(hi[d] >= lo[d])
        if nl >= cap:
            side = 1
        elif nh >= cap:
            side = 0
        elif cl != ch:
            side = 0 if cl < ch else 1
        else:
            side = 0 if nl <= nh else 1
        assign[n] = side
        if side == 0:
            lo[d] += 1
            nl += 1
        else:
            hi[d] += 1
            nh += 1
    return assign


def preprocess(src, dst, n_nodes):
    src = np.asarray(src).astype(np.int64)
    dst = np.asarray(dst).astype(np.int64)
    npc = n_nodes // N_CORES
    NP = ((npc + 127) // 128) * 128
    W = NP // 128
    HALF = 4 * NP
    assert HALF <= 32768, HALF

    assign = _greedy_halves(src, dst, n_nodes)
    degd = np.bincount(dst, minlength=n_nodes)
    core_of = np.empty(n_nodes, dtype=np.int64)
    pos_of = np.empty(n_nodes, dtype=np.int64)
    members = []
    for gstart, base in ((0, 0), (1, 4)):
        mem = np.flatnonzero(assign == gstart)
        mem = mem[np.argsort(-degd[mem], kind="stable")]
        core_of[mem] = base + (np.arange(len(mem)) % 4)
        pos_of[mem] = np.arange(len(mem)) // 4
    for c in range(N_CORES):
        mc = np.flatnonzero(core_of == c)
        members.append(mc[np.argsort(pos_of[mc], kind="stable")])
    core = core_of[dst]
    row_of = core_of * NP + pos_of

    seg_pos = pos_of[dst]
    wv = seg_pos // 128
    pv = seg_pos % 128
    half = (row_of[src] >= HALF).astype(np.int64)

    # dense slot packing: occurrence rank within (core, window, half);
    # any slot may serve any dst of the window (PT matrix in the kernel)
    key = (core * W + wv) * 2 + half
    order = np.argsort(key, kind="stable")
    ks = key[order]
    starts = np.r_[0, np.flatnonzero(np.diff(ks)) + 1]
    gid = np.zeros(len(ks), dtype=np.int64)
    gid[starts[1:]] = 1
    gid = np.cumsum(gid)
    t_in = np.arange(len(ks)) - starts[gid]
    sv = np.empty(len(ks), dtype=np.int64)
    sv[order] = t_in                       # dense slot within (c, w, hf)

    cnt3 = np.bincount(key, minlength=N_CORES * W * 2).reshape(
        N_CORES, W, 2)
    Tm = ((cnt3 + 127) // 128).max(axis=0).astype(np.int64)
    T_lo, T_hi = Tm[:, 0], Tm[:, 1]

    calls = []
    for w in range(W):
        for hf, T in ((0, int(T_lo[w])), (1, int(T_hi[w]))):
            t = 0
            while t < T:
                nt = min(CAP, T - t)
                calls.append((w, hf, nt))
                t += nt
    gtot = int(T_lo.sum() + T_hi.sum())
    icols = 8 * sum(nt for (_, _, nt) in calls)

    tile_off = np.zeros((W, 2), dtype=np.int64)
    acc = 0
    for w in range(W):
        tile_off[w, 0] = acc
        acc += T_lo[w]
        tile_off[w, 1] = acc
        acc += T_hi[w]

    idx_imgs, dstids = [], []
    for c in range(N_CORES):
        m = core == c
        slots_idx = np.zeros((128, gtot), dtype=np.int64)
        slots_did = np.full((128, gtot), 200.0, dtype=np.float32)
        g = tile_off[wv[m], half[m]] + sv[m] // 128
        lane = sv[m] % 128
        slots_idx[lane, g] = row_of[src[m]] - half[m] * HALF
        slots_did[lane, g] = pv[m].astype(np.float32)
        img = np.zeros((16, icols), dtype=np.int16)
        colp = 0
        tile_ptr = {}
        for (w, hf, nt) in calls:
            t0 = tile_ptr.get((w, hf), 0)
            g0 = tile_off[w, hf] + t0
            part = slots_idx[:, g0:g0 + nt]          # [128, nt]
            flat = part.T.reshape(-1)                # j = t*128 + p
            img[:, colp:colp + nt * 8] = flat.reshape(nt * 8, 16).T
            colp += nt * 8
            tile_ptr[(w, hf)] = t0 + nt
        idx_imgs.append(np.ascontiguousarray(np.tile(img, (8, 1))))
        dstids.append(slots_did)

    return dict(members=members, calls=calls, T_lo=T_lo, T_hi=T_hi,
                idx_img=idx_imgs, dstid=dstids, NP=NP, W=W, gtot=gtot,
                icols=icols, npc=npc, HALF=HALF,
                tile_off=tile_off)


def pack_weights(Wl, al, ar):
    """Head-block basis change: y = feat @ M where M's block h has s_h*al_h
    as column 0 (so el_h = y[32h] * al_h[j_h]) and unit columns elsewhere.
    Returns W@M, Minv@A (for er + the C bound), Minv (posthoc undo), and
    crep (the per-head el rescale, replicated to 128 partitions)."""
    H, Dh = Wl.shape[1], Wl.shape[2]
    Wm = np.asarray(Wl, dtype=np.float64).reshape(Wl.shape[0], H * Dh)
    al = np.asarray(al, dtype=np.float64)
    ar = np.asarray(ar, dtype=np.float64)
    M = np.zeros((H * Dh, H * Dh))
    c = np.zeros(H)
    for h in range(H):
        blk = slice(h * Dh, (h + 1) * Dh)
        jh = int(np.argmax(np.abs(al[h])))
        c[h] = al[h][jh]
        Mh = np.zeros((Dh, Dh))
        Mh[:, 0] = al[h] / c[h]
        k = 1
        for j in range(Dh):
            if j != jh:
                Mh[j, k] = 1.0
                k += 1
        M[blk, blk] = Mh
    Minv = np.linalg.inv(M)
    WmM = np.ascontiguousarray((Wm @ M).astype(np.float32))
    A = np.zeros((H * Dh, 8))
    for h in range(H):
        A[h * Dh:(h + 1) * Dh, h] = al[h]
        A[h * Dh:(h + 1) * Dh, 4 + h] = ar[h]
    Ap = np.ascontiguousarray((Minv @ A).astype(np.float32))
    c4 = np.zeros(4, dtype=np.float32)
    c4[:H] = c
    crep = np.ascontiguousarray(np.tile(c4[None, :], (128, 1)))
    Minv128 = np.zeros((128, 128), dtype=np.float32)
    Minv128[:H * Dh, :H * Dh] = Minv
    return WmM, Ap, np.ascontiguousarray(Minv128), crep


# ---------------------------------------------------------------------------
# Device kernel
# ---------------------------------------------------------------------------

def build_nc(meta):
    NP, W, gtot, icols = meta["NP"], meta["W"], meta["gtot"], meta["icols"]
    calls, HALF = meta["calls"], meta["HALF"]
    NTOT = N_CORES * NP
    tile_off = meta["tile_off"]

    nc = bacc.Bacc(None, target_bir_lowering=False, debug=False,
                   num_devices=N_CORES, num_swdge_queues=4)

    hT0 = nc.declare_dram_parameter("hT0", [128, NP], BF16, isOutput=False)
    idx_p = nc.declare_dram_parameter("idx", [128, icols], I16, isOutput=False)
    did_p = nc.declare_dram_parameter("dstid", [128, gtot], BF16,
                                      isOutput=False)
    iotab_p = nc.declare_dram_parameter("iotab", [128, 128], BF16,
                                        isOutput=False)
    Wp = [nc.declare_dram_parameter(f"W{l}", [128, 128], BF16, isOutput=False)
          for l in range(3)]
    Ap = [nc.declare_dram_parameter(f"A{l}", [128, 8], BF16, isOutput=False)
          for l in range(3)]
    Mp = [nc.declare_dram_parameter(f"Minv{l}", [128, 128], BF16,
                                    isOutput=False) for l in range(3)]
    Cp = [nc.declare_dram_parameter(f"crep{l}", [128, 4], F32,
                                    isOutput=False) for l in range(3)]
    ident_p = nc.declare_dram_parameter("ident", [128, 128], F32,
                                        isOutput=False)
    identb_p = nc.declare_dram_parameter("identb", [128, 128], BF16,
                                         isOutput=False)
    ones_p = nc.declare_dram_parameter("ones1", [1, 128], F32, isOutput=False)
    onescol_p = nc.declare_dram_parameter("onescol", [128, 1], F32,
                                          isOutput=False)
    out_p = nc.declare_dram_parameter("out", [NP, 128], F32, isOutput=True)

    with tile.TileContext(nc) as tc:
        with (
            tc.tile_pool(name="const", bufs=1) as constp,
            tc.tile_pool(name="persist", bufs=1) as pers,
            tc.tile_pool(name="featg", bufs=12) as fgp,
            tc.tile_pool(name="mext", bufs=8) as mxp,
            tc.tile_pool(name="ptpool", bufs=8) as ptp,
            tc.tile_pool(name="small", bufs=6) as smp,
            tc.tile_pool(name="psum", bufs=3, space="PSUM") as psp,
            tc.tile_pool(name="psacc", bufs=2, space="PSUM") as psaccp,
            tc.tile_pool(name="dram", bufs=1, space="DRAM") as dramp,
        ):
            ident = constp.tile([128, 128], F32, tag="ident")
            nc.sync.dma_start(ident[:], ident_p[:, :])
            identb = constp.tile([128, 128], BF16, tag="identb")
            nc.sync.dma_start(identb[:], identb_p[:, :])
            ones1 = constp.tile([1, 128], F32, tag="ones1")
            nc.sync.dma_start(ones1[:], ones_p[:, :])
            onescol = constp.tile([128, 1], F32, tag="onescol")
            nc.sync.dma_start(onescol[:], onescol_p[:, :])
            Wt = [constp.tile([128, 128], BF16, tag=f"W{l}", name=f"Wt{l}")
                  for l in range(3)]
            At = [constp.tile([128, 8], BF16, tag=f"A{l}", name=f"At{l}")
                  for l in range(3)]
            Mt = [constp.tile([128, 128], BF16, tag=f"Minv{l}",
                              name=f"Mt{l}") for l in range(3)]
            Ct = [constp.tile([128, 4], F32, tag=f"crep{l}", name=f"Ct{l}")
                  for l in range(3)]
            for l in range(3):
                nc.sync.dma_start(Wt[l][:], Wp[l][:, :])
                nc.sync.dma_start(At[l][:], Ap[l][:, :])
                nc.sync.dma_start(Mt[l][:], Mp[l][:, :])
                nc.sync.dma_start(Ct[l][:], Cp[l][:, :])
            idx_sb = pers.tile([128, icols], I16, tag="idx")
            nc.sync.dma_start(idx_sb[:], idx_p[:, :])
            dstid_sb = pers.tile([128, gtot], BF16, tag="dstid")
            nc.sync.dma_start(dstid_sb[:], did_p[:, :])
            iotab = constp.tile([128, 128], BF16, tag="iotab")
            nc.sync.dma_start(iotab[:], iotab_p[:, :])

            hT = [pers.tile([128, W, 128], BF16, tag=f"hT{i}", name=f"hT{i}")
                  for i in range(2)]
            nc.sync.dma_start(hT[0][:, :, :],
                              hT0[:, :].rearrange("p (w n) -> p w n", w=W))

            elerB = pers.tile([128, W, 8], F32, tag="elerB")
            erBb = pers.tile([128, W, 4], BF16, tag="erBb")
            rowimg = pers.tile([128, W, ROW], BF16, tag="rowimg")

            loc_tbl = dramp.tile([NP, ROW], BF16, tag="loctbl")
            full2 = [dramp.tile([NTOT, ROW], BF16, tag=f"fulltbl{i}",
                                name=f"full{i}") for i in range(2)]

            def phase_a_window(lyr, w):
                Hh = HEADS[lyr]
                hc = hT[lyr % 2]
                featT_ps = psp.tile([128, 128], F32, tag="ps",
                                    name="featT_ps")
                nc.tensor.matmul(featT_ps[:], Wt[lyr][:],
                                 hc[:, w, :], start=True, stop=True)
                featT_sb = smp.tile([128, 128], BF16, tag="featT_sb",
                                    name="featT_sb")
                nc.vector.tensor_copy(featT_sb[:], featT_ps[:])
                elerT_ps = psp.tile([8, 128], F32, tag="ps", name="elerT_ps")
                nc.tensor.matmul(elerT_ps[:], At[lyr][:], featT_sb[:],
                                 start=True, stop=True)
                elerT_sb = smp.tile([8, 128], F32, tag="elerT_sb",
                                    name="elerT_sb")
                nc.vector.tensor_copy(elerT_sb[:], elerT_ps[:])
                eler_ps = psp.tile([128, 8], F32, tag="ps", name="eler_ps")
                nc.tensor.matmul(eler_ps[:], elerT_sb[:],
                                 ident[0:8, 0:8], is_transpose=True,
                                 start=True, stop=True)
                nc.vector.tensor_copy(elerB[:, w, :], eler_ps[:])
                nc.vector.tensor_copy(erBb[:, w, 0:Hh],
                                      eler_ps[:, 4:4 + Hh])
                feat_ps = psp.tile([128, 128], BF16, tag="psb",
                                   name="feat_ps")
                nc.tensor.matmul(feat_ps[:], featT_sb[:], identb[:, :],
                                 is_transpose=True, start=True, stop=True)
                nc.vector.tensor_copy(rowimg[:, w, :], feat_ps[:])
                nc.sync.dma_start(
                    loc_tbl[:].rearrange("(w p) f -> w p f", p=128)
                    [w, :, :],
                    rowimg[:, w, :])

            def ag_chunk(lyr, w):
                a, b = (w - 6) * 128, (w + 1) * 128
                nc.gpsimd.collective_compute(
                    "AllGather", OP.bypass,
                    replica_groups=[list(range(N_CORES))],
                    ins=[loc_tbl[a:b, :].opt()],
                    outs=[full2[lyr % 2]
                          .rearrange("(c r) f -> c r f", c=N_CORES)
                          [:, a:b, :].opt()])

            for w in range(W):
                phase_a_window(0, w)
                if w % 7 == 6:
                    ag_chunk(0, w)

            for layer in range(3):
                H = HEADS[layer]
                D = 128 // H
                hcur, hnext = hT[layer % 2], hT[(layer + 1) % 2]

                # ---- -C = -(lrelu(max el + max er) + margin) ----
                mx = smp.tile([128, 2], F32, tag="mx")
                nc.vector.tensor_reduce(mx[:, 0:1], elerB[:, :, 0:H],
                                        axis=AX.XY, op=OP.max)
                nc.vector.tensor_reduce(mx[:, 1:2], elerB[:, :, 4:4 + H],
                                        axis=AX.XY, op=OP.max)
                mxT_ps = psp.tile([2, 128], F32, tag="ps")
                nc.tensor.matmul(mxT_ps[:], mx[:], ident[:, :],
                                 is_transpose=True, start=True, stop=True)
                mm = smp.tile([2, 1], F32, tag="mm")
                nc.vector.tensor_reduce(mm[:], mxT_ps[:, :], axis=AX.X,
                                        op=OP.max)
                s_ps = psp.tile([1, 1], F32, tag="ps")
                nc.tensor.matmul(s_ps[:], mm[:], onescol[0:2, 0:1],
                                 start=True, stop=True)
                cs = smp.tile([1, 4], F32, tag="cs")
                nc.vector.tensor_copy(cs[:, 0:1], s_ps[:])
                nc.vector.tensor_scalar(cs[:, 1:2], cs[:, 0:1], NEG_SLOPE,
                                        None, op0=OP.mult)
                nc.vector.tensor_tensor(cs[:, 2:3], cs[:, 0:1],
                                        cs[:, 1:2], op=OP.max)
                nc.vector.tensor_scalar(cs[:, 3:4], cs[:, 2:3], -1.0,
                                        -C_MARGIN, op0=OP.mult,
                                        op1=OP.add)
                negC_ps = psp.tile([128, 1], F32, tag="ps")
                nc.tensor.matmul(negC_ps[:], ones1[:], cs[:, 3:4],
                                 start=True, stop=True)
                negC = smp.tile([128, 1], F32, tag="negC")
                nc.vector.tensor_copy(negC[:], negC_ps[:])

                # ======== Phase B ========
                tbl_lo = full2[layer % 2][0:HALF, :]
                tbl_hi = full2[layer % 2][HALF:NTOT, :]
                colp = 0
                tile_ptr = {}
                cur_w = -1
                acc_ps = None
                first_mm = True
                ntiles_w = {w: int(meta["T_lo"][w] + meta["T_hi"][w])
                            for w in range(W)}
                done_w = {w: 0 for w in range(W)}
                qn = 0
                for (w, hf, nt) in calls:
                    if w != cur_w:
                        cur_w = w
                        acc_ps = psaccp.tile([128, 132], F32, tag="acc")
                        first_mm = True
                    t0 = tile_ptr.get((w, hf), 0)
                    tile_ptr[(w, hf)] = t0 + nt
                    g0 = int(tile_off[w, hf]) + t0

                    fg = fgp.tile([128, CAP, ROW], BF16, tag="fg")
                    src_ap = tbl_lo if hf == 0 else tbl_hi
                    nc.gpsimd.dma_gather(
                        fg[:, 0:nt, :], src_ap,
                        idx_sb[:, colp:colp + nt * 8],
                        nt * 128, nt * 128, ROW, elem_step=ROW,
                        single_packet=False, queue_num=qn)
                    qn = (qn + 1) % 4
                    colp += nt * 8

                    t = 0
                    while t < nt:
                        g = min(8, nt - t)
                        # PT[slot, dst] = (dstid[slot] == dst), per tile
                        pt4 = ptp.tile([128, 8, 128], BF16, tag="pt4")
                        did_b = (dstid_sb[:, g0 + t:g0 + t + g].unsqueeze(2)
                                 .broadcast_to([128, g, 128]))
                        iot_b = (iotab[:].unsqueeze(1)
                                 .broadcast_to([128, g, 128]))
                        nc.vector.tensor_tensor(pt4[:, 0:g, :], did_b,
                                                iot_b, op=OP.is_equal)
                        # er aligned to slots: er_slot = PT^T-permute of er
                        er4_ps = psp.tile([128, 8, 4], F32, tag="ps")
                        for k in range(g):
                            ptT_ps = psp.tile([128, 128], BF16, tag="psb")
                            nc.tensor.matmul(ptT_ps[:], pt4[:, k, :],
                                             identb[:, :], is_transpose=True,
                                             start=True, stop=True)
                            ptT_sb = smp.tile([128, 128], BF16, tag="ptT")
                            nc.scalar.activation(ptT_sb[:], ptT_ps[:],
                                                 AF.Copy)
                            nc.tensor.matmul(er4_ps[:, k, 0:H], ptT_sb[:],
                                             erBb[:, w, 0:H],
                                             start=True, stop=True)
                        sx = smp.tile([128, 8, 4], F32, tag="sx")
                        ux = smp.tile([128, 8, 4], F32, tag="ux")
                        ex = smp.tile([128, 8, 4], F32, tag="exx")
                        elg = smp.tile([128, 8, 4], F32, tag="elg")
                        c_b = (Ct[layer][:, 0:H].unsqueeze(1)
                               .broadcast_to([128, g, H]).unsqueeze(3))
                        nc.vector.tensor_tensor(
                            elg[:, 0:g, 0:H].unsqueeze(3),
                            fg[:, t:t + g, :].rearrange(
                                "p g (h d) -> p g h d", h=H)[:, :, :, 0:1],
                            c_b, op=OP.mult)
                        nc.vector.tensor_tensor(
                            sx[:, 0:g, 0:H], elg[:, 0:g, 0:H],
                            er4_ps[:, 0:g, 0:H], op=OP.add)
                        nc.scalar.activation(ux[:, 0:g, 0:H], sx[:, 0:g, 0:H],
                                             AF.Exp, bias=negC[:, 0:1],
                                             scale=1.0)
                        nc.scalar.activation(ex[:, 0:g, 0:H], sx[:, 0:g, 0:H],
                                             AF.Exp, bias=negC[:, 0:1],
                                             scale=NEG_SLOPE)
                        mext = mxp.tile([128, 8, 132], BF16, tag="mext")
                        nc.vector.scalar_tensor_tensor(
                            mext[:, 0:g, 128:128 + H], ux[:, 0:g, 0:H], 1.0,
                            ex[:, 0:g, 0:H], op0=OP.mult, op1=OP.max)
                        ex_b = (mext[:, 0:g, 128:128 + H].unsqueeze(3)
                                .broadcast_to([128, g, H, D]))
                        nc.vector.tensor_tensor(
                            mext[:, 0:g, 0:128]
                            .rearrange("p g (h d) -> p g h d", h=H),
                            fg[:, t:t + g, 0:128]
                            .rearrange("p g (h d) -> p g h d", h=H),
                            ex_b, op=OP.mult)
                        for k in range(g):
                            done_w[w] += 1
                            nc.tensor.matmul(
                                acc_ps[:, 0:128 + H], pt4[:, k, :],
                                mext[:, k, 0:128 + H],
                                start=first_mm,
                                stop=(done_w[w] == ntiles_w[w]))
                            first_mm = False
                        t += g

                    if done_w[w] == ntiles_w[w]:
                        dn = smp.tile([128, 8], F32, tag="dn")
                        nc.vector.tensor_scalar(dn[:, 0:H],
                                                acc_ps[:, 128:128 + H],
                                                1e-9, None, op0=OP.add)
                        nc.vector.reciprocal(dn[:, 4:4 + H], dn[:, 0:H])
                        rec_b = (dn[:, 4:4 + H].unsqueeze(2)
                                 .broadcast_to([128, H, D]))
                        hsb = smp.tile([128, 128], BF16, tag="hsb")
                        nc.vector.tensor_tensor(
                            hsb[:].rearrange("p (h d) -> p h d", h=H),
                            acc_ps[:, 0:128]
                            .rearrange("p (h d) -> p h d", h=H),
                            rec_b, op=OP.mult)
                        hT_ps = psp.tile([128, 128], BF16, tag="psb")
                        nc.tensor.matmul(hT_ps[:], hsb[:], identb[:, :],
                                         is_transpose=True,
                                         start=True, stop=True)
                        hTsb = smp.tile([128, 128], BF16, tag="hTsb")
                        nc.vector.tensor_copy(hTsb[:], hT_ps[:])
                        if layer < 2:
                            # hT_next = Minv^T @ hsb^T (undo basis), then relu
                            hT2_ps = psp.tile([128, 128], F32, tag="ps")
                            nc.tensor.matmul(hT2_ps[:], Mt[layer][:],
                                             hTsb[:], start=True, stop=True)
                            nc.scalar.activation(hnext[:, w, :], hT2_ps[:],
                                                 AF.Relu)
                        else:
                            # out = hsb @ Minv (node-major)
                            out_ps = psp.tile([128, 128], F32, tag="ps")
                            nc.tensor.matmul(out_ps[:], hTsb[:],
                                             Mt[layer][:], start=True,
                                             stop=True)
                            osb = smp.tile([128, 128], F32, tag="osb")
                            nc.vector.tensor_copy(osb[:], out_ps[:])
                            nc.sync.dma_start(
                                out_p[:, :].rearrange("(w p) f -> w p f",
                                                      p=128)[w, :, :],
                                osb[:])
                        if layer < 2:
                            phase_a_window(layer + 1, w)
                            if w % 7 == 6:
                                ag_chunk(layer + 1, w)
    nc.finalize()
    return nc


# ---------------------------------------------------------------------------
# Entry point
# ---------------------------------------------------------------------------

def kernel(features, src, dst, W0, al0, ar0, W1, al1, ar1, W2, al2, ar2):
    out, _ = run_gat(features, src, dst, W0, al0, ar0, W1, al1, ar1,
                     W2, al2, ar2, trace=False)
    return out


def run_gat(features, src, dst, W0, al0, ar0, W1, al1, ar1, W2, al2, ar2,
            trace=False):
    features = np.asarray(features, dtype=np.float32)
    n_nodes = features.shape[0]
    meta = preprocess(src, dst, n_nodes)
    NP, W, npc = meta["NP"], meta["W"], meta["npc"]

    Wm0, A0, Mi0, C0 = pack_weights(np.asarray(W0), al0, ar0)
    Wm1, A1, Mi1, C1 = pack_weights(np.asarray(W1), al1, ar1)
    Wm2, A2, Mi2, C2 = pack_weights(np.asarray(W2), al2, ar2)

    ident = np.eye(128, dtype=np.float32)
    identb = np.eye(128, dtype=np.float32).astype(BF)
    ones1 = np.ones((1, 128), dtype=np.float32)
    onescol = np.ones((128, 1), dtype=np.float32)

    in_maps = []
    for c in range(N_CORES):
        mem = meta["members"][c]
        h_c = np.zeros((NP, 128), dtype=np.float32)
        h_c[:len(mem)] = features[mem]
        in_maps.append({
            "hT0": np.ascontiguousarray(h_c.T).astype(BF),
            "idx": meta["idx_img"][c],
            "dstid": meta["dstid"][c].astype(BF),
            "iotab": np.tile(np.arange(128, dtype=np.float32)[None, :],
                             (128, 1)).astype(BF),
            "W0": Wm0.astype(BF), "W1": Wm1.astype(BF), "W2": Wm2.astype(BF),
            "A0": A0.astype(BF), "A1": A1.astype(BF), "A2": A2.astype(BF),
            "Minv0": Mi0.astype(BF), "Minv1": Mi1.astype(BF),
            "Minv2": Mi2.astype(BF),
            "crep0": C0, "crep1": C1, "crep2": C2,
            "ident": ident, "identb": identb, "ones1": ones1,
            "onescol": onescol,
        })

    nc = build_nc(meta)
    br = run_bass_kernel_spmd(nc, in_maps, list(range(N_CORES)), trace=trace)
    res = br.results

    out = np.empty((n_nodes, 128), dtype=np.float32)
    for c in range(N_CORES):
        mem = meta["members"][c]
        o = np.asarray(res[c]["out"])
        out[mem] = o[:len(mem)]
    return out, br
